# revision 16
# baseline (speedup 1.0000x reference)
"""Trainium2 Bass kernel for GQA attention (B=4, T=1024, D=4096, 32 Q heads,
8 KV heads, RoPE, full softmax attention, output projection).

Sharding: 8 cores = 4 batches x 2 query-blocks of 512 tokens. Each core
computes K/V for the 512 tokens of its own block (pairs of cores that share
a batch exchange halves via 2-rank AllGathers) and runs attention + output
projection for its 512 queries.

Token order per core is host-rotated so the core's query block is always
tokens [0:512) -- full (maskless) attention is permutation-invariant in the
key/value tokens, so each core runs an identical SPMD program.

Datapath: x and all weights are cast to bf16 on the host (halves HBM
traffic) and pre-packed into SBUF-layout [128, N] panels so every weight
DMA is a single contiguous ~1 MiB transfer; all matmuls are bf16 with f32
PSUM accumulation. Per head the softmax denominator comes from a
ones-vector matmul accumulated alongside PV; its reciprocal is broadcast
to 128 partitions via a rank-1 bf16 matmul. Score matmuls+exp are
interleaved into the next head's Q-projection stream so the ScalarE exp
cascade never stalls the PE.

DMA queues: weight/activation loads ride the sync HWDGE ring, SBUF->DRAM
stores ride the scalar HWDGE ring, and the collectives plus gathered K/V
loads ride the gpsimd SWDGE ring, so no compute stream ever queues behind
a collective.
"""

import sys
import math

import numpy as np

if "/opt/trn_rl_repo" not in sys.path:
    sys.path.insert(0, "/opt/trn_rl_repo")

HEAD_DIM = 128
N_HEADS = 32
N_KV = 8
B, S, K_POS, D = 4, 32, 32, 4096
T = S * K_POS          # 1024 tokens per batch
QB = 512               # queries per core
N_CORES = 8
SCALE = HEAD_DIM ** -0.5
DT = D // 128          # 32 d-tiles
LAG = 4                # attention trails Q-projection by LAG heads

_CACHE = {}


def _install_tile_drain_fix():
    """walrus in this image rejects >1 sem wait on one CTRL (Drain)
    instruction; spread the Tile tail-drain waits across sync-engine NOPs."""
    import concourse.tile as tile_mod
    import concourse.mybir as mybir
    from concourse.vector_clock import ScopedClock

    if getattr(tile_mod.TileContext, "_drain_fix_installed", False):
        return

    def _patched(self, tick_clock, wait_clock):
        nc = self.nc
        drain_inst = nc.sync.drain()
        wait_clock.add_sem_waits(
            drain_inst.ins, ScopedClock({None: tick_clock.global_clock})
        )
        si = drain_inst.ins.sync_info
        waits = list(si.on_wait) if si is not None and si.on_wait else []
        if len(waits) > 1:
            si.on_wait = waits[:1]
            for w in waits[1:]:
                nop = nc.sync.nop(nofuse=True)
                nop.ins.sync_info = mybir.SyncInfo(on_wait=[w], on_update=[])
        nc.all_engine_barrier()
        assert self.sems is not None
        popped = nc._tile_sem_poison_stack.pop()
        assert popped is self._sem_poison
        nc.clear_and_free_semaphores(list(self.sems.allocated().values()))
        nc.all_engine_barrier()

    tile_mod.TileContext._drain_and_barrier = _patched
    tile_mod.TileContext._drain_fix_installed = True


def _split_multi_waits(nc, mybir):
    """walrus here rejects >1 sem wait per instruction: hoist extra waits
    onto same-engine NOPs inserted immediately before the instruction."""
    import copy

    template = None
    for fn in nc.m.functions:
        for bb in fn.blocks:
            for inst in bb.instructions:
                if type(inst).__name__ == "InstNoOp":
                    template = inst
                    break
            if template is not None:
                break
    assert template is not None, "no InstNoOp template found"

    n_added = 0
    for fn in nc.m.functions:
        for bb in fn.blocks:
            new_list = []
            changed = False
            for inst in bb.instructions:
                si = inst.sync_info
                waits = list(si.on_wait) if si is not None and si.on_wait else []
                if len(waits) > 1:
                    changed = True
                    for w in waits[:-1]:
                        nop = copy.deepcopy(template)
                        nop.name = f"I-wsplit-{nc.next_id()}"
                        nop.engine = inst.engine
                        nop.sync_info = mybir.SyncInfo(on_wait=[w], on_update=[])
                        nc.register_instruction(nop, overwrite=True)
                        new_list.append(nop)
                        n_added += 1
                    si.on_wait = waits[-1:]
                new_list.append(inst)
            if changed:
                bb.instructions = new_list
    return n_added


def _rope_emit(nc, pool, ps, dst, cos2, sin2, f32):
    """ps: [128, 512] psum (rows 0:64 = even/'real' dims, 64:128 = odd);
    dst: [128, 512] bf16 sbuf. cos2/sin2: [128, 512] with both halves equal
    to cos(f)/sin(f)."""
    a = pool.tile([128, QB], f32, name="rpA", tag="rpA")
    bs = pool.tile([128, QB], f32, name="rpB", tag="rpB")
    nc.vector.tensor_mul(a[:], ps[:], cos2[:])
    nc.vector.tensor_mul(bs[0:64, :], ps[64:128, :], sin2[64:128, :])
    nc.vector.tensor_mul(bs[64:128, :], ps[0:64, :], sin2[0:64, :])
    nc.vector.tensor_sub(dst[0:64, :], a[0:64, :], bs[0:64, :])
    nc.vector.tensor_add(dst[64:128, :], a[64:128, :], bs[64:128, :])


def _build():
    import concourse.bass as bass
    import concourse.mybir as mybir
    import concourse.tile as tile

    _install_tile_drain_fix()

    f32 = mybir.dt.float32
    bf16 = mybir.dt.bfloat16
    Sin = mybir.ActivationFunctionType.Sin
    Exp = mybir.ActivationFunctionType.Exp

    nc = bass.Bass("TRN2", target_bir_lowering=False, debug=False)

    xA = nc.declare_dram_parameter("xA", [128, DT * QB], bf16, isOutput=False)
    fqT = nc.declare_dram_parameter("fqT", [64, QB], f32, isOutput=False)
    wq4 = nc.declare_dram_parameter("wq4", [N_HEADS, 128, D], bf16, isOutput=False)
    wk4 = nc.declare_dram_parameter("wk4", [N_KV, 128, D], bf16, isOutput=False)
    wvA = nc.declare_dram_parameter("wvA", [128, DT * 1024], bf16, isOutput=False)
    wo4 = nc.declare_dram_parameter("wo4", [8, 128, D * 4], bf16, isOutput=False)
    out = nc.declare_dram_parameter("out", [QB, D], f32, isOutput=True)

    rg = [[0, 1], [2, 3], [4, 5], [6, 7]]

    with tile.TileContext(nc) as tc:
        with tc.tile_pool(name="const", bufs=1) as constp:
            # ---- resident tiles (alloc order = reverse release order) ----
            attp = tc.alloc_tile_pool(name="attn", bufs=1)
            attn_sb = [attp.tile([128, QB], bf16, name=f"at{h}")
                       for h in range(N_HEADS)]
            vp = tc.alloc_tile_pool(name="vsb", bufs=1)
            kp = tc.alloc_tile_pool(name="ksb", bufs=1)
            v_sb = [vp.tile([128, T], bf16, name=f"v{kt}") for kt in range(8)]
            k_sb = [kp.tile([128, T], bf16, name=f"k{kh}") for kh in range(N_KV)]
            xqp = tc.alloc_tile_pool(name="xqp", bufs=1)
            xq_all = xqp.tile([128, DT * QB], bf16, name="xq_all")

            def load_xa(j):
                nc.sync.dma_start(
                    out=xq_all[:, j * 4096:(j + 1) * 4096],
                    in_=xA.ap()[:, j * 4096:(j + 1) * 4096])

            def xq_sl(d):
                return xq_all[:, d * QB:(d + 1) * QB]

            wkp = tc.alloc_tile_pool(name="wkp", bufs=2)
            wqp = tc.alloc_tile_pool(name="wqp", bufs=3)
            wop = tc.alloc_tile_pool(name="wop", bufs=2)
            wk_tiles, wq_tiles, wo_tiles = {}, {}, {}

            def emit_wk(kh):
                t = wkp.tile([128, D], bf16, name="wk_sl", tag="wk_sl")
                nc.sync.dma_start(out=t[:], in_=wk4.ap()[kh])
                wk_tiles[kh] = t

            def emit_wq(h):
                t = wqp.tile([128, D], bf16, name="wq_sl", tag="wq_sl")
                nc.sync.dma_start(out=t[:], in_=wq4.ap()[h])
                wq_tiles[h] = t

            def emit_wo(i):
                db, q4 = i // 4, i % 4
                t = wop.tile([128, 8 * QB], bf16, name="wo_sl", tag="wo_sl")
                nc.sync.dma_start(
                    out=t[:], in_=wo4.ap()[db][:, q4 * 4096:(q4 + 1) * 4096])
                wo_tiles[i] = t

            load_xa(0)

            with tc.tile_pool(name="dramb", bufs=1, space="DRAM") as dramp:
                v_half = [dramp.tile([4, 128, QB], bf16, name=f"v_half{fb}")
                          for fb in range(2)]
                v_gath = [dramp.tile([2, 4, 128, QB], bf16, name=f"v_gath{fb}")
                          for fb in range(2)]
                k_half = [dramp.tile([4, 128, QB], bf16, name=f"k_half{i}")
                          for i in range(2)]
                k_gath = [dramp.tile([2, 4, 128, QB], bf16, name=f"k_gath{i}")
                          for i in range(2)]

                # ---- V projection (own 512 tokens): ps[fb*4+tt] = [tok, feat]
                with tc.tile_pool(name="wvp", bufs=3) as wvp, \
                     tc.tile_pool(name="vstg", bufs=1) as vstg, \
                     tc.tile_pool(name="psv", bufs=1, space="PSUM") as psv:
                    wv_tiles = {}

                    def emit_wv(j):
                        t = wvp.tile([128, 4096], bf16, name="wv", tag="wv")
                        nc.sync.dma_start(
                            out=t[:], in_=wvA.ap()[:, j * 4096:(j + 1) * 4096])
                        wv_tiles[j] = t

                    emit_wv(0)

                    # ---- sincos: freqs in [0, 2pi), Sin accepts [-pi, pi]:
                    #   sin(t) = sin(pi - t); cos(t) = 1 - 2*sin(t/2)^2
                    fq_sb = constp.tile([64, QB], f32, name="fq_sb")
                    nc.sync.dma_start(out=fq_sb[:], in_=fqT.ap())
                    load_xa(1)
                    emit_wv(1)
                    load_xa(2)
                    emit_wv(2)
                    load_xa(3)
                    emit_wk(0)
                    cos2 = constp.tile([128, QB], f32, name="cos2")
                    sin2 = constp.tile([128, QB], f32, name="sin2")
                    pi_ap = constp.tile([64, 1], f32, name="pi_ap")
                    nc.vector.memset(pi_ap[:], math.pi)
                    s_half = constp.tile([64, QB], f32, name="s_half")
                    nc.scalar.activation(s_half[:], fq_sb[:], Sin,
                                         bias=0.0, scale=0.5)
                    sq = constp.tile([64, QB], f32, name="sq")
                    nc.vector.tensor_mul(sq[:], s_half[:], s_half[:])
                    for half in (0, 64):
                        nc.vector.tensor_scalar(
                            cos2[half:half + 64, :], sq[:], -2.0, 1.0,
                            mybir.AluOpType.mult, mybir.AluOpType.add)
                        nc.scalar.activation(
                            sin2[half:half + 64, :], fq_sb[:], Sin,
                            bias=pi_ap[:], scale=-1.0)
                    # preload the ScalarE Exp table off the critical path
                    warm = constp.tile([1, 1], f32, name="warm")
                    nc.scalar.activation(warm[:], pi_ap[0:1, 0:1], Exp,
                                         bias=0.0, scale=0.0)

                    ps = [psv.tile([128, QB], f32, name=f"psv{i}",
                                   tag=f"psv{i}") for i in range(8)]
                    for d in range(DT):
                        if d % 4 == 0 and d // 4 + 3 < 8:
                            emit_wv(d // 4 + 3)
                        wv_d = wv_tiles[d // 4]
                        base = (d % 4) * 1024
                        for fb in range(2):
                            for tt in range(4):
                                nc.tensor.matmul(
                                    ps[fb * 4 + tt][:],
                                    lhsT=xq_sl(d)[:, tt * 128:(tt + 1) * 128],
                                    rhs=wv_d[:, base + fb * QB:
                                             base + (fb + 1) * QB],
                                    start=(d == 0), stop=(d == DT - 1))
                        if d // 4 - 1 in wv_tiles and d % 4 == 3:
                            wv_tiles.pop(d // 4 - 1, None)
                    for fb in range(2):
                        for tt in range(4):
                            vs = vstg.tile([128, QB], bf16, name="vs",
                                           tag=f"vs{fb}{tt}")
                            if tt % 2 == 0:
                                nc.vector.tensor_copy(vs[:], ps[fb * 4 + tt][:])
                            else:
                                nc.scalar.copy(vs[:], ps[fb * 4 + tt][:])
                            nc.scalar.dma_start(out=v_half[fb][tt], in_=vs[:])
                        nc.gpsimd.collective_compute(
                            "AllGather", mybir.AluOpType.bypass,
                            ins=[v_half[fb].opt()], outs=[v_gath[fb].opt()],
                            replica_groups=rg)

                # ---- K projection (own 512 tokens) + RoPE ----
                with tc.tile_pool(name="kstg", bufs=2) as kstg, \
                     tc.tile_pool(name="ropek", bufs=2) as ropek, \
                     tc.tile_pool(name="psk", bufs=2, space="PSUM") as psk:
                    for kh in range(N_KV):
                        if kh + 1 < N_KV:
                            emit_wk(kh + 1)
                        if kh >= 5:
                            emit_wq(kh - 5)
                        wk_sl = wk_tiles.pop(kh)
                        pk = psk.tile([128, QB], f32, name="pk", tag="pk")
                        for d in range(DT):
                            nc.tensor.matmul(
                                pk[:],
                                lhsT=wk_sl[:, d * 128:(d + 1) * 128],
                                rhs=xq_sl(d),
                                start=(d == 0), stop=(d == DT - 1))
                        ks = kstg.tile([128, QB], bf16, name="ks", tag="ks")
                        _rope_emit(nc, ropek, pk, ks[:], cos2, sin2, f32)
                        nc.scalar.dma_start(out=k_half[kh // 4][kh % 4],
                                            in_=ks[:])
                        if kh % 4 == 3:
                            nc.gpsimd.collective_compute(
                                "AllGather", mybir.AluOpType.bypass,
                                ins=[k_half[kh // 4].opt()],
                                outs=[k_gath[kh // 4].opt()],
                                replica_groups=rg)

                # gathered K/V -> SBUF (gpsimd ring, behind the collectives)
                for kt in range(8):
                    for fb in range(2):
                        nc.gpsimd.dma_start(
                            out=v_sb[kt][:, fb * QB:(fb + 1) * QB],
                            in_=v_gath[fb][kt // 4, kt % 4])
                for half in range(2):
                    for rr in range(2):
                        for j in range(4):
                            kh = half * 4 + j
                            nc.gpsimd.dma_start(
                                out=k_sb[kh][:, rr * QB:(rr + 1) * QB],
                                in_=k_gath[half][rr, j])

                # ---- Q projection + attention, software-pipelined ----
                _q_attention(nc, tc, mybir, xq_sl, k_sb, v_sb, cos2, sin2,
                             attn_sb, wq_tiles, emit_wq, emit_wo)
                _out_proj(nc, tc, mybir, out, attn_sb, wo_tiles, emit_wo)
                wop.release()
                wqp.release()
                wkp.release()
                xqp.release()
                kp.release()
                vp.release()
                attp.release()

    _split_multi_waits(nc, mybir)
    return nc


def _q_attention(nc, tc, mybir, xq_sl, k_sb, v_sb, cos2, sin2, attn_sb,
                 wq_tiles, emit_wq, emit_wo):
    f32 = mybir.dt.float32
    bf16 = mybir.dt.bfloat16
    Exp = mybir.ActivationFunctionType.Exp

    with tc.tile_pool(name="qsb", bufs=5) as qsb, \
         tc.tile_pool(name="ropeq", bufs=1) as ropeq, \
         tc.tile_pool(name="ptil", bufs=2) as ptp, \
         tc.tile_pool(name="gsum", bufs=1) as gsp, \
         tc.tile_pool(name="ptsum", bufs=2) as tsp, \
         tc.tile_pool(name="pvsb", bufs=3) as pvp, \
         tc.tile_pool(name="rsb", bufs=2) as rsbp, \
         tc.tile_pool(name="pss", bufs=2, space="PSUM") as pss, \
         tc.tile_pool(name="ppv", bufs=1, space="PSUM") as ppv, \
         tc.tile_pool(name="psq", bufs=2, space="PSUM") as psq, \
         tc.tile_pool(name="pden", bufs=2, space="PSUM") as pden, \
         tc.tile_pool(name="prb", bufs=1, space="PSUM") as prb:

        # constant [128,1] / [1,128] ones for the den / broadcast matmuls
        ones_col = qsb.tile([128, 1], bf16, name="ones_col", bufs=1)
        nc.vector.memset(ones_col[:], 1.0)
        ones_row = qsb.tile([1, 128], bf16, name="ones_row", bufs=1)
        nc.vector.memset(ones_row[:], 1.0)

        q_tiles = {}
        St = {}   # a -> stage state dict

        def emit_score_kt(a, kt):
            kh = a // 4
            ps_s = pss.tile([128, QB], f32, name="ps_s", tag="ps_s")
            nc.tensor.matmul(
                ps_s[:], lhsT=k_sb[kh][:, kt * 128:(kt + 1) * 128],
                rhs=q_tiles[a][:], start=True, stop=True)
            nc.scalar.activation(
                St[a]["pt"][:, kt * QB:(kt + 1) * QB], ps_s[:],
                Exp, bias=0.0, scale=SCALE)

        def stage_a_pe(a):
            # PV accumulation (single PSUM bank; evacuated by ScalarE below)
            kh = a // 4
            pt = St[a]["pt"]
            pv = ppv.tile([128, QB], f32, name="pv", tag="pv")
            for kt in range(8):
                nc.tensor.matmul(
                    pv[:], lhsT=v_sb[kt][:, kh * 128:(kh + 1) * 128],
                    rhs=pt[:, kt * QB:(kt + 1) * QB],
                    start=(kt == 0), stop=(kt == 7))
            St[a]["pv"] = pv

        def stage_a_post(a, drain=False):
            # evacuate PV; GpSimd: fold the 8 key-tiles of exp down to 2
            pt = St[a]["pt"]
            pv_sb = pvp.tile([128, QB], bf16, name="pv_sb", tag="pv_sb")
            if drain:
                nc.vector.tensor_copy(pv_sb[:], St[a]["pv"][:])
            else:
                nc.scalar.copy(pv_sb[:], St[a]["pv"][:])
            s1 = gsp.tile([128, 4 * QB], bf16, name="dfold", tag="dfold")
            nc.gpsimd.tensor_add(s1[:], pt[:, 0:4 * QB], pt[:, 4 * QB:8 * QB])
            ptsum = tsp.tile([128, 2 * QB], bf16, name="ptsum", tag="ptsum")
            nc.gpsimd.tensor_add(ptsum[:], s1[:, 0:2 * QB],
                                 s1[:, 2 * QB:4 * QB])
            St[a]["pv_sb"] = pv_sb
            St[a]["ptsum"] = ptsum

        def stage_b_pe(a):
            # den[1,512] = ones.T @ ptsum  (partition reduction on PE)
            den = pden.tile([1, QB], f32, name="den", tag="den")
            ptsum = St[a]["ptsum"]
            nc.tensor.matmul(den[:], lhsT=ones_col[:], rhs=ptsum[:, 0:QB],
                             start=True, stop=False)
            nc.tensor.matmul(den[:], lhsT=ones_col[:], rhs=ptsum[:, QB:2 * QB],
                             start=False, stop=True)
            St[a]["den"] = den

        def stage_b_dve(a):
            recip = rsbp.tile([1, QB], bf16, name="recip", tag="recip")
            with nc.allow_low_precision(reason="softmax denom in bf16"):
                nc.vector.reciprocal(recip[:], St[a]["den"][:])
            St[a]["recip"] = recip

        def stage_c_pe(a):
            # broadcast 1/den to 128 partitions (rank-1 bf16 matmul)
            ps_rb = prb.tile([128, QB], f32, name="ps_rb", tag="ps_rb")
            nc.tensor.matmul(ps_rb[:], lhsT=ones_row[:],
                             rhs=St[a]["recip"][:], start=True, stop=True)
            St[a]["ps_rb"] = ps_rb

        def stage_c_post(a):
            rb_sb = rsbp.tile([128, QB], f32, name="rb_sb", tag="rb_sb")
            nc.scalar.copy(rb_sb[:], St[a]["ps_rb"][:])
            nc.vector.tensor_mul(attn_sb[a][:], St[a]["pv_sb"][:], rb_sb[:])

        for h in range(N_HEADS):
            a, b, c = h - LAG, h - LAG - 1, h - LAG - 2
            a = a if a >= 0 else None
            b = b if b >= 0 else None
            c = c if c >= 0 else None

            if h + 3 < N_HEADS:
                emit_wq(h + 3)
            if a is not None:
                St[a] = {"pt": ptp.tile([128, 8 * QB], bf16, name="pt",
                                        tag="pt")}

            ps_q = psq.tile([128, QB], f32, name="ps_q", tag="ps_q")
            wq_sl = wq_tiles.pop(h)
            for d in range(DT):
                nc.tensor.matmul(
                    ps_q[:], lhsT=wq_sl[:, d * 128:(d + 1) * 128],
                    rhs=xq_sl(d), start=(d == 0), stop=(d == DT - 1))
                if a is not None and d % 4 == 3:
                    emit_score_kt(a, d // 4)

            if b is not None:
                stage_b_pe(b)
            if c is not None:
                stage_c_pe(c)
            if a is not None:
                stage_a_pe(a)

            if c is not None:
                stage_c_post(c)
            q_t = qsb.tile([128, QB], bf16, name="q_t", tag="q_t")
            _rope_emit(nc, ropeq, ps_q, q_t[:], cos2, sin2, f32)
            q_tiles[h] = q_t
            if b is not None:
                stage_b_dve(b)
            if a is not None:
                stage_a_post(a)
                q_tiles.pop(a, None)
            if c is not None:
                del St[c]

        # ---- drain: heads 28..31 (scores pair-interleaved for ACT pacing),
        # then flush the b/c stages ----
        emit_wo(0)
        A0, A1 = N_HEADS - LAG, N_HEADS - LAG + 1   # 28, 29
        A2, A3 = N_HEADS - LAG + 2, N_HEADS - LAG + 3
        for a in (A0, A1):
            St[a] = {"pt": ptp.tile([128, 8 * QB], bf16, name="pt", tag="pt")}
        for kt in range(8):
            emit_score_kt(A0, kt)
            emit_score_kt(A1, kt)
        stage_b_pe(A0 - 1)
        stage_c_pe(A0 - 2)
        stage_a_pe(A0)
        stage_a_pe(A1)
        stage_c_post(A0 - 2)
        stage_b_dve(A0 - 1)
        stage_a_post(A0, drain=True)
        stage_a_post(A1, drain=True)
        q_tiles.pop(A0, None)
        q_tiles.pop(A1, None)
        del St[A0 - 2]

        for a in (A2, A3):
            St[a] = {"pt": ptp.tile([128, 8 * QB], bf16, name="pt", tag="pt")}
        for kt in range(8):
            emit_score_kt(A2, kt)
            emit_score_kt(A3, kt)
        stage_b_pe(A0)
        stage_b_pe(A1)
        stage_c_pe(A0 - 1)
        stage_a_pe(A2)
        stage_a_pe(A3)
        stage_c_post(A0 - 1)
        stage_b_dve(A0)
        stage_b_dve(A1)
        stage_a_post(A2, drain=True)
        stage_a_post(A3, drain=True)
        q_tiles.pop(A2, None)
        q_tiles.pop(A3, None)
        del St[A0 - 1]

        stage_b_pe(A2)
        stage_b_pe(A3)
        stage_c_pe(A0)
        stage_c_pe(A1)
        stage_c_post(A0)
        stage_c_post(A1)
        stage_b_dve(A2)
        stage_b_dve(A3)
        del St[A0], St[A1]

        stage_c_pe(A2)
        stage_c_pe(A3)
        stage_c_post(A2)
        stage_c_post(A3)
        del St[A2], St[A3]


def _out_proj(nc, tc, mybir, out, attn_sb, wo_tiles, emit_wo):
    f32 = mybir.dt.float32
    with tc.tile_pool(name="psout", bufs=2, space="PSUM") as psout, \
         tc.tile_pool(name="ostg", bufs=4) as ostg:

        for db in range(8):
            po = [psout.tile([128, QB], f32, name=f"po{qt}", tag=f"po{qt}")
                  for qt in range(4)]
            for q4 in range(4):
                i = db * 4 + q4
                if i + 1 < 32:
                    emit_wo(i + 1)
                wo_sl = wo_tiles.pop(i)
                for f8 in range(8):
                    f = q4 * 8 + f8
                    for qt in range(4):
                        nc.tensor.matmul(
                            po[qt][:],
                            lhsT=attn_sb[f][:, qt * 128:(qt + 1) * 128],
                            rhs=wo_sl[:, f8 * QB:(f8 + 1) * QB],
                            start=(f == 0), stop=(f == 31))
            for qt in range(4):
                o_stg = ostg.tile([128, QB], f32, name="o_stg", tag="o_stg")
                nc.vector.tensor_copy(o_stg[:], po[qt][:])
                nc.scalar.dma_start(
                    out=out.ap()[qt * 128:(qt + 1) * 128,
                                 db * QB:(db + 1) * QB],
                    in_=o_stg[:])


def _prep_shards(x, freqs, wq, wk, wv, wo):
    """Host-side sharding + layout prep + bf16 cast (numpy only)."""
    import ml_dtypes
    bf = ml_dtypes.bfloat16

    rope_perm = np.concatenate([np.arange(0, HEAD_DIM, 2),
                                np.arange(1, HEAD_DIM, 2)])
    f_perm_q = np.concatenate([h * HEAD_DIM + rope_perm for h in range(N_HEADS)])
    f_perm_k = np.concatenate([h * HEAD_DIM + rope_perm for h in range(N_KV)])

    wqT_p = np.ascontiguousarray(wq[f_perm_q].T)     # [D, 4096]
    wkT_p = np.ascontiguousarray(wk[f_perm_k].T)     # [D, 1024]
    wvT = np.ascontiguousarray(wv.T)                 # [D, 1024]
    woT = wo.T                                        # [F, D]

    # wq4[h, p, d*128+c] = wqT_p[d*128+p, h*128+c]
    wq4 = np.ascontiguousarray(
        wqT_p.reshape(DT, 128, N_HEADS, 128).transpose(2, 1, 0, 3)
        .reshape(N_HEADS, 128, D)).astype(bf)
    wk4 = np.ascontiguousarray(
        wkT_p.reshape(DT, 128, N_KV, 128).transpose(2, 1, 0, 3)
        .reshape(N_KV, 128, D)).astype(bf)
    # wvA[p, d*1024+f] = wvT[d*128+p, f]
    wvA = np.ascontiguousarray(
        wvT.reshape(DT, 128, N_KV * 128).transpose(1, 0, 2)
        .reshape(128, DT * 1024)).astype(bf)
    # wo4[db, fp, ft*512+c] = woT[ft*128+fp, db*512+c]
    wo4 = np.ascontiguousarray(
        woT.reshape(DT, 128, 8, 512).transpose(2, 1, 0, 3)
        .reshape(8, 128, D * 4)).astype(bf)

    fq_flat = freqs.reshape(T, HEAD_DIM // 2)

    in_maps = []
    for c in range(N_CORES):
        b, qb = c // 2, c % 2
        qoff = qb * QB
        perm = np.arange(qoff, qoff + QB)
        xb = x[b].reshape(T, D)[perm]
        xT = np.ascontiguousarray(xb.T)              # [D, QB]
        # xA[p, d*512+c] = xT[d*128+p, c]
        xAc = np.ascontiguousarray(
            xT.reshape(DT, 128, QB).transpose(1, 0, 2)
            .reshape(128, DT * QB)).astype(bf)
        in_maps.append({
            "xA": xAc,
            "fqT": np.ascontiguousarray(fq_flat[perm].T),
            "wq4": wq4,
            "wk4": wk4,
            "wvA": wvA,
            "wo4": wo4,
        })
    return in_maps


def kernel(x, freqs, wq, wk, wv, wo, _trace=False, _trace_kwargs=None):
    from concourse.bass_utils import run_bass_kernel_spmd

    x = np.asarray(x, dtype=np.float32)
    freqs = np.asarray(freqs, dtype=np.float32)
    wq = np.asarray(wq, dtype=np.float32)
    wk = np.asarray(wk, dtype=np.float32)
    wv = np.asarray(wv, dtype=np.float32)
    wo = np.asarray(wo, dtype=np.float32)

    if "nc" not in _CACHE:
        _CACHE["nc"] = _build()
    nc = _CACHE["nc"]

    in_maps = _prep_shards(x, freqs, wq, wk, wv, wo)
    res = run_bass_kernel_spmd(
        nc, in_maps, core_ids=list(range(N_CORES)), trace=_trace,
        **(_trace_kwargs or {}))
    _CACHE["last_result"] = res

    full = np.zeros((B, T, D), np.float32)
    for c in range(N_CORES):
        b, qb = c // 2, c % 2
        full[b, qb * QB:(qb + 1) * QB, :] = res.results[c]["out"]
    return full.reshape(B, S, K_POS, D)


# revision 17
# speedup vs baseline: 1.1795x; 1.1795x over previous
"""Trainium2 Bass kernel for GQA attention (B=4, T=1024, D=4096, 32 Q heads,
8 KV heads, RoPE, full softmax attention, output projection).

Sharding: 8 cores = 4 batches x 2 query-blocks of 512 tokens. Each core
computes K/V for the 512 tokens of its own block (pairs of cores that share
a batch exchange halves via 2-rank AllGathers) and runs attention + output
projection for its 512 queries.

Token order per core is host-rotated so the core's query block is always
tokens [0:512) -- full (maskless) attention is permutation-invariant in the
key/value tokens, so each core runs an identical SPMD program.

Datapath: x and all weights are cast to bf16 on the host (halves HBM
traffic) and pre-packed into SBUF-layout [128, N] panels so every weight
DMA is a single contiguous ~1 MiB transfer; all matmuls are bf16 with f32
PSUM accumulation. Per head the softmax denominator comes from a
ones-vector matmul accumulated alongside PV; its reciprocal is broadcast
to 128 partitions via a rank-1 bf16 matmul. Score matmuls+exp are
interleaved into the next head's Q-projection stream so the ScalarE exp
cascade never stalls the PE.

DMA queues: weight/activation loads ride the sync HWDGE ring, SBUF->DRAM
stores ride the scalar HWDGE ring, and the collectives plus gathered K/V
loads ride the gpsimd SWDGE ring, so no compute stream ever queues behind
a collective.
"""

import sys
import math

import numpy as np

if "/opt/trn_rl_repo" not in sys.path:
    sys.path.insert(0, "/opt/trn_rl_repo")

HEAD_DIM = 128
N_HEADS = 32
N_KV = 8
B, S, K_POS, D = 4, 32, 32, 4096
T = S * K_POS          # 1024 tokens per batch
QB = 512               # queries per core
N_CORES = 8
SCALE = HEAD_DIM ** -0.5
DT = D // 128          # 32 d-tiles
LAG = 4                # attention trails Q-projection by LAG heads

_CACHE = {}


def _install_tile_drain_fix():
    """walrus in this image rejects >1 sem wait on one CTRL (Drain)
    instruction; spread the Tile tail-drain waits across sync-engine NOPs."""
    import concourse.tile as tile_mod
    import concourse.mybir as mybir
    from concourse.vector_clock import ScopedClock

    if getattr(tile_mod.TileContext, "_drain_fix_installed", False):
        return

    def _patched(self, tick_clock, wait_clock):
        nc = self.nc
        drain_inst = nc.sync.drain()
        wait_clock.add_sem_waits(
            drain_inst.ins, ScopedClock({None: tick_clock.global_clock})
        )
        si = drain_inst.ins.sync_info
        waits = list(si.on_wait) if si is not None and si.on_wait else []
        if len(waits) > 1:
            si.on_wait = waits[:1]
            for w in waits[1:]:
                nop = nc.sync.nop(nofuse=True)
                nop.ins.sync_info = mybir.SyncInfo(on_wait=[w], on_update=[])
        nc.all_engine_barrier()
        assert self.sems is not None
        popped = nc._tile_sem_poison_stack.pop()
        assert popped is self._sem_poison
        nc.clear_and_free_semaphores(list(self.sems.allocated().values()))
        nc.all_engine_barrier()

    tile_mod.TileContext._drain_and_barrier = _patched
    tile_mod.TileContext._drain_fix_installed = True


def _split_multi_waits(nc, mybir):
    """walrus here rejects >1 sem wait per instruction: hoist extra waits
    onto same-engine NOPs inserted immediately before the instruction."""
    import copy

    template = None
    for fn in nc.m.functions:
        for bb in fn.blocks:
            for inst in bb.instructions:
                if type(inst).__name__ == "InstNoOp":
                    template = inst
                    break
            if template is not None:
                break
    assert template is not None, "no InstNoOp template found"

    n_added = 0
    for fn in nc.m.functions:
        for bb in fn.blocks:
            new_list = []
            changed = False
            for inst in bb.instructions:
                si = inst.sync_info
                waits = list(si.on_wait) if si is not None and si.on_wait else []
                if len(waits) > 1:
                    changed = True
                    for w in waits[:-1]:
                        nop = copy.deepcopy(template)
                        nop.name = f"I-wsplit-{nc.next_id()}"
                        nop.engine = inst.engine
                        nop.sync_info = mybir.SyncInfo(on_wait=[w], on_update=[])
                        nc.register_instruction(nop, overwrite=True)
                        new_list.append(nop)
                        n_added += 1
                    si.on_wait = waits[-1:]
                new_list.append(inst)
            if changed:
                bb.instructions = new_list
    return n_added


def _rope_emit(nc, pool, ps, dst, cos2, sin2, f32):
    """ps: [128, 512] psum (rows 0:64 = even/'real' dims, 64:128 = odd);
    dst: [128, 512] bf16 sbuf. cos2/sin2: [128, 512] with both halves equal
    to cos(f)/sin(f)."""
    a = pool.tile([128, QB], f32, name="rpA", tag="rpA")
    bs = pool.tile([128, QB], f32, name="rpB", tag="rpB")
    nc.vector.tensor_mul(a[:], ps[:], cos2[:])
    nc.vector.tensor_mul(bs[0:64, :], ps[64:128, :], sin2[64:128, :])
    nc.vector.tensor_mul(bs[64:128, :], ps[0:64, :], sin2[0:64, :])
    nc.vector.tensor_sub(dst[0:64, :], a[0:64, :], bs[0:64, :])
    nc.vector.tensor_add(dst[64:128, :], a[64:128, :], bs[64:128, :])


def _build():
    import concourse.bass as bass
    import concourse.mybir as mybir
    import concourse.tile as tile

    _install_tile_drain_fix()

    f32 = mybir.dt.float32
    bf16 = mybir.dt.bfloat16
    Sin = mybir.ActivationFunctionType.Sin
    Exp = mybir.ActivationFunctionType.Exp

    nc = bass.Bass("TRN2", target_bir_lowering=False, debug=False)

    xA = nc.declare_dram_parameter("xA", [128, DT * QB], bf16, isOutput=False)
    fqT = nc.declare_dram_parameter("fqT", [64, QB], f32, isOutput=False)
    wq4 = nc.declare_dram_parameter("wq4", [N_HEADS, 128, D], bf16, isOutput=False)
    wk4 = nc.declare_dram_parameter("wk4", [N_KV, 128, D], bf16, isOutput=False)
    wvA = nc.declare_dram_parameter("wvA", [128, DT * 1024], bf16, isOutput=False)
    wo4 = nc.declare_dram_parameter("wo4", [8, 128, D * 4], bf16, isOutput=False)
    out = nc.declare_dram_parameter("out", [QB, D], f32, isOutput=True)

    rg = [[0, 1], [2, 3], [4, 5], [6, 7]]

    with tile.TileContext(nc) as tc:
        with tc.tile_pool(name="const", bufs=1) as constp:
            # ---- resident tiles (alloc order = reverse release order) ----
            attp = tc.alloc_tile_pool(name="attn", bufs=1)
            attn_sb = [attp.tile([128, QB], bf16, name=f"at{h}")
                       for h in range(N_HEADS)]
            vp = tc.alloc_tile_pool(name="vsb", bufs=1)
            kp = tc.alloc_tile_pool(name="ksb", bufs=1)
            v_sb = [vp.tile([128, T], bf16, name=f"v{kt}") for kt in range(8)]
            k_sb = [kp.tile([128, T], bf16, name=f"k{kh}") for kh in range(N_KV)]
            xqp = tc.alloc_tile_pool(name="xqp", bufs=1)
            xq_all = xqp.tile([128, DT * QB], bf16, name="xq_all")

            def load_xa(j):
                nc.sync.dma_start(
                    out=xq_all[:, j * 4096:(j + 1) * 4096],
                    in_=xA.ap()[:, j * 4096:(j + 1) * 4096])

            def xq_sl(d):
                return xq_all[:, d * QB:(d + 1) * QB]

            wkp = tc.alloc_tile_pool(name="wkp", bufs=2)
            wqp = tc.alloc_tile_pool(name="wqp", bufs=3)
            wop = tc.alloc_tile_pool(name="wop", bufs=2)
            wk_tiles, wq_tiles, wo_tiles = {}, {}, {}

            def emit_wk(kh):
                t = wkp.tile([128, D], bf16, name="wk_sl", tag="wk_sl")
                nc.sync.dma_start(out=t[:], in_=wk4.ap()[kh])
                wk_tiles[kh] = t

            def emit_wq(h):
                t = wqp.tile([128, D], bf16, name="wq_sl", tag="wq_sl")
                nc.sync.dma_start(out=t[:], in_=wq4.ap()[h])
                wq_tiles[h] = t

            def emit_wo(i):
                db, q4 = i // 4, i % 4
                t = wop.tile([128, 8 * QB], bf16, name="wo_sl", tag="wo_sl")
                nc.sync.dma_start(
                    out=t[:], in_=wo4.ap()[db][:, q4 * 4096:(q4 + 1) * 4096])
                wo_tiles[i] = t

            load_xa(0)

            with tc.tile_pool(name="dramb", bufs=1, space="DRAM") as dramp:
                v_half = [dramp.tile([4, 128, QB], bf16, name=f"v_half{fb}")
                          for fb in range(2)]
                v_gath = [dramp.tile([2, 4, 128, QB], bf16, name=f"v_gath{fb}")
                          for fb in range(2)]
                k_half = [dramp.tile([4, 128, QB], bf16, name=f"k_half{i}")
                          for i in range(2)]
                k_gath = [dramp.tile([2, 4, 128, QB], bf16, name=f"k_gath{i}")
                          for i in range(2)]

                # ---- V projection (own 512 tokens): ps[fb*4+tt] = [tok, feat]
                with tc.tile_pool(name="wvp", bufs=3) as wvp, \
                     tc.tile_pool(name="vstg", bufs=1) as vstg, \
                     tc.tile_pool(name="psv", bufs=1, space="PSUM") as psv:
                    wv_tiles = {}

                    def emit_wv(j):
                        t = wvp.tile([128, 4096], bf16, name="wv", tag="wv")
                        nc.sync.dma_start(
                            out=t[:], in_=wvA.ap()[:, j * 4096:(j + 1) * 4096])
                        wv_tiles[j] = t

                    emit_wv(0)

                    # ---- sincos: freqs in [0, 2pi), Sin accepts [-pi, pi]:
                    #   sin(t) = sin(pi - t); cos(t) = 1 - 2*sin(t/2)^2
                    fq_sb = constp.tile([64, QB], f32, name="fq_sb")
                    nc.sync.dma_start(out=fq_sb[:], in_=fqT.ap())
                    load_xa(1)
                    emit_wv(1)
                    load_xa(2)
                    emit_wv(2)
                    load_xa(3)
                    emit_wk(0)
                    cos2 = constp.tile([128, QB], f32, name="cos2")
                    sin2 = constp.tile([128, QB], f32, name="sin2")
                    pi_ap = constp.tile([64, 1], f32, name="pi_ap")
                    nc.vector.memset(pi_ap[:], math.pi)
                    s_half = constp.tile([64, QB], f32, name="s_half")
                    nc.scalar.activation(s_half[:], fq_sb[:], Sin,
                                         bias=0.0, scale=0.5)
                    sq = constp.tile([64, QB], f32, name="sq")
                    nc.vector.tensor_mul(sq[:], s_half[:], s_half[:])
                    for half in (0, 64):
                        nc.vector.tensor_scalar(
                            cos2[half:half + 64, :], sq[:], -2.0, 1.0,
                            mybir.AluOpType.mult, mybir.AluOpType.add)
                        nc.scalar.activation(
                            sin2[half:half + 64, :], fq_sb[:], Sin,
                            bias=pi_ap[:], scale=-1.0)
                    # preload the ScalarE Exp table off the critical path
                    warm = constp.tile([1, 1], f32, name="warm")
                    nc.scalar.activation(warm[:], pi_ap[0:1, 0:1], Exp,
                                         bias=0.0, scale=0.0)

                    ps = [psv.tile([128, QB], f32, name=f"psv{i}",
                                   tag=f"psv{i}") for i in range(8)]
                    for d in range(DT):
                        if d % 4 == 0 and d // 4 + 3 < 8:
                            emit_wv(d // 4 + 3)
                        wv_d = wv_tiles[d // 4]
                        base = (d % 4) * 1024
                        for fb in range(2):
                            for tt in range(4):
                                nc.tensor.matmul(
                                    ps[fb * 4 + tt][:],
                                    lhsT=xq_sl(d)[:, tt * 128:(tt + 1) * 128],
                                    rhs=wv_d[:, base + fb * QB:
                                             base + (fb + 1) * QB],
                                    start=(d == 0), stop=(d == DT - 1))
                        if d // 4 - 1 in wv_tiles and d % 4 == 3:
                            wv_tiles.pop(d // 4 - 1, None)
                    for fb in range(2):
                        for tt in range(4):
                            vs = vstg.tile([128, QB], bf16, name="vs",
                                           tag=f"vs{fb}{tt}")
                            if tt % 2 == 0:
                                nc.vector.tensor_copy(vs[:], ps[fb * 4 + tt][:])
                            else:
                                nc.scalar.copy(vs[:], ps[fb * 4 + tt][:])
                            nc.scalar.dma_start(out=v_half[fb][tt], in_=vs[:])
                        nc.gpsimd.collective_compute(
                            "AllGather", mybir.AluOpType.bypass,
                            ins=[v_half[fb].opt()], outs=[v_gath[fb].opt()],
                            replica_groups=rg)

                # ---- K projection (own 512 tokens) + RoPE ----
                with tc.tile_pool(name="kstg", bufs=2) as kstg, \
                     tc.tile_pool(name="ropek", bufs=2) as ropek, \
                     tc.tile_pool(name="psk", bufs=2, space="PSUM") as psk:
                    for kh in range(N_KV):
                        if kh + 1 < N_KV:
                            emit_wk(kh + 1)
                        if kh >= 5:
                            emit_wq(kh - 5)
                        wk_sl = wk_tiles.pop(kh)
                        pk = psk.tile([128, QB], f32, name="pk", tag="pk")
                        for d in range(DT):
                            nc.tensor.matmul(
                                pk[:],
                                lhsT=wk_sl[:, d * 128:(d + 1) * 128],
                                rhs=xq_sl(d),
                                start=(d == 0), stop=(d == DT - 1))
                        ks = kstg.tile([128, QB], bf16, name="ks", tag="ks")
                        _rope_emit(nc, ropek, pk, ks[:], cos2, sin2, f32)
                        nc.scalar.dma_start(out=k_half[kh // 4][kh % 4],
                                            in_=ks[:])
                        if kh % 4 == 3:
                            nc.gpsimd.collective_compute(
                                "AllGather", mybir.AluOpType.bypass,
                                ins=[k_half[kh // 4].opt()],
                                outs=[k_gath[kh // 4].opt()],
                                replica_groups=rg)

                # gathered K/V -> SBUF (gpsimd ring, behind the collectives)
                for kt in range(8):
                    for fb in range(2):
                        nc.gpsimd.dma_start(
                            out=v_sb[kt][:, fb * QB:(fb + 1) * QB],
                            in_=v_gath[fb][kt // 4, kt % 4])
                for half in range(2):
                    for rr in range(2):
                        for j in range(4):
                            kh = half * 4 + j
                            nc.gpsimd.dma_start(
                                out=k_sb[kh][:, rr * QB:(rr + 1) * QB],
                                in_=k_gath[half][rr, j])

                # ---- Q projection + attention, software-pipelined ----
                _q_attention(nc, tc, mybir, xq_sl, k_sb, v_sb, cos2, sin2,
                             attn_sb, wq_tiles, emit_wq, emit_wo)
                _out_proj(nc, tc, mybir, out, attn_sb, wo_tiles, emit_wo)
                wop.release()
                wqp.release()
                wkp.release()
                xqp.release()
                kp.release()
                vp.release()
                attp.release()

    _split_multi_waits(nc, mybir)
    return nc


def _q_attention(nc, tc, mybir, xq_sl, k_sb, v_sb, cos2, sin2, attn_sb,
                 wq_tiles, emit_wq, emit_wo):
    f32 = mybir.dt.float32
    bf16 = mybir.dt.bfloat16
    Exp = mybir.ActivationFunctionType.Exp

    with tc.tile_pool(name="qsb", bufs=5) as qsb, \
         tc.tile_pool(name="ropeq", bufs=1) as ropeq, \
         tc.tile_pool(name="ptil", bufs=2) as ptp, \
         tc.tile_pool(name="gsum", bufs=1) as gsp, \
         tc.tile_pool(name="ptsum", bufs=2) as tsp, \
         tc.tile_pool(name="pvsb", bufs=3) as pvp, \
         tc.tile_pool(name="rsb", bufs=2) as rsbp, \
         tc.tile_pool(name="pss", bufs=2, space="PSUM") as pss, \
         tc.tile_pool(name="ppv", bufs=1, space="PSUM") as ppv, \
         tc.tile_pool(name="psq", bufs=2, space="PSUM") as psq, \
         tc.tile_pool(name="pden", bufs=2, space="PSUM") as pden, \
         tc.tile_pool(name="prb", bufs=1, space="PSUM") as prb:

        # constant [128,1] / [1,128] ones for the den / broadcast matmuls
        ones_col = qsb.tile([128, 1], bf16, name="ones_col", bufs=1)
        nc.vector.memset(ones_col[:], 1.0)
        ones_row = qsb.tile([1, 128], bf16, name="ones_row", bufs=1)
        nc.vector.memset(ones_row[:], 1.0)

        q_tiles = {}
        St = {}   # a -> stage state dict

        def emit_score_kt(a, kt):
            kh = a // 4
            ps_s = pss.tile([128, QB], f32, name="ps_s", tag="ps_s")
            nc.tensor.matmul(
                ps_s[:], lhsT=k_sb[kh][:, kt * 128:(kt + 1) * 128],
                rhs=q_tiles[a][:], start=True, stop=True)
            nc.scalar.activation(
                St[a]["pt"][:, kt * QB:(kt + 1) * QB], ps_s[:],
                Exp, bias=0.0, scale=SCALE)

        def stage_a_pe(a):
            # PV accumulation (single PSUM bank; evacuated by ScalarE below)
            kh = a // 4
            pt = St[a]["pt"]
            pv = ppv.tile([128, QB], f32, name="pv", tag="pv")
            for kt in range(8):
                nc.tensor.matmul(
                    pv[:], lhsT=v_sb[kt][:, kh * 128:(kh + 1) * 128],
                    rhs=pt[:, kt * QB:(kt + 1) * QB],
                    start=(kt == 0), stop=(kt == 7))
            St[a]["pv"] = pv

        def stage_a_post(a, drain=False):
            # evacuate PV; GpSimd: fold the 8 key-tiles of exp down to 2
            pt = St[a]["pt"]
            pv_sb = pvp.tile([128, QB], bf16, name="pv_sb", tag="pv_sb")
            if drain:
                nc.vector.tensor_copy(pv_sb[:], St[a]["pv"][:])
            else:
                nc.scalar.copy(pv_sb[:], St[a]["pv"][:])
            s1 = gsp.tile([128, 2 * QB], bf16, name="dfold", tag="dfold")
            nc.gpsimd.tensor_add(s1[:], pt[:, 0:2 * QB], pt[:, 2 * QB:4 * QB])
            nc.gpsimd.tensor_add(s1[:, 0:QB], s1[:, 0:QB], s1[:, QB:2 * QB])
            ptsum = tsp.tile([128, QB], bf16, name="ptsum", tag="ptsum")
            nc.gpsimd.tensor_add(ptsum[:], pt[:, 4 * QB:5 * QB],
                                 pt[:, 5 * QB:6 * QB])
            nc.gpsimd.tensor_add(ptsum[:], ptsum[:], pt[:, 6 * QB:7 * QB])
            nc.gpsimd.tensor_add(ptsum[:], ptsum[:], pt[:, 7 * QB:8 * QB])
            nc.gpsimd.tensor_add(ptsum[:], ptsum[:], s1[:, 0:QB])
            St[a]["pv_sb"] = pv_sb
            St[a]["ptsum"] = ptsum

        def stage_b_pe(a):
            # den[1,512] = ones.T @ ptsum  (partition reduction on PE)
            den = pden.tile([1, QB], f32, name="den", tag="den")
            nc.tensor.matmul(den[:], lhsT=ones_col[:], rhs=St[a]["ptsum"][:],
                             start=True, stop=True)
            St[a]["den"] = den

        def stage_b_dve(a):
            recip = rsbp.tile([1, QB], bf16, name="recip", tag="recip")
            with nc.allow_low_precision(reason="softmax denom in bf16"):
                nc.vector.reciprocal(recip[:], St[a]["den"][:])
            St[a]["recip"] = recip

        def stage_c_pe(a):
            # broadcast 1/den to 128 partitions (rank-1 bf16 matmul)
            ps_rb = prb.tile([128, QB], f32, name="ps_rb", tag="ps_rb")
            nc.tensor.matmul(ps_rb[:], lhsT=ones_row[:],
                             rhs=St[a]["recip"][:], start=True, stop=True)
            St[a]["ps_rb"] = ps_rb

        def stage_c_post(a):
            rb_sb = rsbp.tile([128, QB], f32, name="rb_sb", tag="rb_sb")
            nc.scalar.copy(rb_sb[:], St[a]["ps_rb"][:])
            nc.vector.tensor_mul(attn_sb[a][:], St[a]["pv_sb"][:], rb_sb[:])

        for h in range(N_HEADS):
            a, b, c = h - LAG, h - LAG - 1, h - LAG - 2
            a = a if a >= 0 else None
            b = b if b >= 0 else None
            c = c if c >= 0 else None

            if h + 3 < N_HEADS:
                emit_wq(h + 3)
            if a is not None:
                St[a] = {"pt": ptp.tile([128, 8 * QB], bf16, name="pt",
                                        tag="pt")}

            ps_q = psq.tile([128, QB], f32, name="ps_q", tag="ps_q")
            wq_sl = wq_tiles.pop(h)
            for d in range(DT):
                nc.tensor.matmul(
                    ps_q[:], lhsT=wq_sl[:, d * 128:(d + 1) * 128],
                    rhs=xq_sl(d), start=(d == 0), stop=(d == DT - 1))
                if a is not None and d % 4 == 3:
                    emit_score_kt(a, d // 4)

            if b is not None:
                stage_b_pe(b)
            if c is not None:
                stage_c_pe(c)
            if a is not None:
                stage_a_pe(a)

            if c is not None:
                stage_c_post(c)
            q_t = qsb.tile([128, QB], bf16, name="q_t", tag="q_t")
            _rope_emit(nc, ropeq, ps_q, q_t[:], cos2, sin2, f32)
            q_tiles[h] = q_t
            if b is not None:
                stage_b_dve(b)
            if a is not None:
                stage_a_post(a)
                q_tiles.pop(a, None)
            if c is not None:
                del St[c]

        # ---- drain: heads 28..31 (scores pair-interleaved for ACT pacing),
        # then flush the b/c stages ----
        emit_wo(0)
        A0, A1 = N_HEADS - LAG, N_HEADS - LAG + 1   # 28, 29
        A2, A3 = N_HEADS - LAG + 2, N_HEADS - LAG + 3
        for a in (A0, A1):
            St[a] = {"pt": ptp.tile([128, 8 * QB], bf16, name="pt", tag="pt")}
        for kt in range(8):
            emit_score_kt(A0, kt)
            emit_score_kt(A1, kt)
        stage_b_pe(A0 - 1)
        stage_c_pe(A0 - 2)
        stage_a_pe(A0)
        stage_a_pe(A1)
        stage_c_post(A0 - 2)
        stage_b_dve(A0 - 1)
        stage_a_post(A0, drain=True)
        stage_a_post(A1, drain=True)
        q_tiles.pop(A0, None)
        q_tiles.pop(A1, None)
        del St[A0 - 2]

        for a in (A2, A3):
            St[a] = {"pt": ptp.tile([128, 8 * QB], bf16, name="pt", tag="pt")}
        for kt in range(8):
            emit_score_kt(A2, kt)
            emit_score_kt(A3, kt)
        stage_b_pe(A0)
        stage_b_pe(A1)
        stage_c_pe(A0 - 1)
        stage_a_pe(A2)
        stage_a_pe(A3)
        stage_c_post(A0 - 1)
        stage_b_dve(A0)
        stage_b_dve(A1)
        stage_a_post(A2, drain=True)
        stage_a_post(A3, drain=True)
        q_tiles.pop(A2, None)
        q_tiles.pop(A3, None)
        del St[A0 - 1]

        stage_b_pe(A2)
        stage_b_pe(A3)
        stage_c_pe(A0)
        stage_c_pe(A1)
        stage_c_post(A0)
        stage_c_post(A1)
        stage_b_dve(A2)
        stage_b_dve(A3)
        del St[A0], St[A1]

        stage_c_pe(A2)
        stage_c_pe(A3)
        stage_c_post(A2)
        stage_c_post(A3)
        del St[A2], St[A3]


def _out_proj(nc, tc, mybir, out, attn_sb, wo_tiles, emit_wo):
    f32 = mybir.dt.float32
    with tc.tile_pool(name="psout", bufs=2, space="PSUM") as psout, \
         tc.tile_pool(name="ostg", bufs=4) as ostg:

        for db in range(8):
            po = [psout.tile([128, QB], f32, name=f"po{qt}", tag=f"po{qt}")
                  for qt in range(4)]
            for q4 in range(4):
                i = db * 4 + q4
                if i + 1 < 32:
                    emit_wo(i + 1)
                wo_sl = wo_tiles.pop(i)
                for f8 in range(8):
                    f = q4 * 8 + f8
                    for qt in range(4):
                        nc.tensor.matmul(
                            po[qt][:],
                            lhsT=attn_sb[f][:, qt * 128:(qt + 1) * 128],
                            rhs=wo_sl[:, f8 * QB:(f8 + 1) * QB],
                            start=(f == 0), stop=(f == 31))
            for qt in range(4):
                o_stg = ostg.tile([128, QB], f32, name="o_stg", tag="o_stg")
                nc.vector.tensor_copy(o_stg[:], po[qt][:])
                nc.scalar.dma_start(
                    out=out.ap()[qt * 128:(qt + 1) * 128,
                                 db * QB:(db + 1) * QB],
                    in_=o_stg[:])


def _prep_shards(x, freqs, wq, wk, wv, wo):
    """Host-side sharding + layout prep + bf16 cast (numpy only)."""
    import ml_dtypes
    bf = ml_dtypes.bfloat16

    rope_perm = np.concatenate([np.arange(0, HEAD_DIM, 2),
                                np.arange(1, HEAD_DIM, 2)])
    f_perm_q = np.concatenate([h * HEAD_DIM + rope_perm for h in range(N_HEADS)])
    f_perm_k = np.concatenate([h * HEAD_DIM + rope_perm for h in range(N_KV)])

    wqT_p = np.ascontiguousarray(wq[f_perm_q].T)     # [D, 4096]
    wkT_p = np.ascontiguousarray(wk[f_perm_k].T)     # [D, 1024]
    wvT = np.ascontiguousarray(wv.T)                 # [D, 1024]
    woT = wo.T                                        # [F, D]

    # wq4[h, p, d*128+c] = wqT_p[d*128+p, h*128+c]
    wq4 = np.ascontiguousarray(
        wqT_p.reshape(DT, 128, N_HEADS, 128).transpose(2, 1, 0, 3)
        .reshape(N_HEADS, 128, D)).astype(bf)
    wk4 = np.ascontiguousarray(
        wkT_p.reshape(DT, 128, N_KV, 128).transpose(2, 1, 0, 3)
        .reshape(N_KV, 128, D)).astype(bf)
    # wvA[p, d*1024+f] = wvT[d*128+p, f]
    wvA = np.ascontiguousarray(
        wvT.reshape(DT, 128, N_KV * 128).transpose(1, 0, 2)
        .reshape(128, DT * 1024)).astype(bf)
    # wo4[db, fp, ft*512+c] = woT[ft*128+fp, db*512+c]
    wo4 = np.ascontiguousarray(
        woT.reshape(DT, 128, 8, 512).transpose(2, 1, 0, 3)
        .reshape(8, 128, D * 4)).astype(bf)

    fq_flat = freqs.reshape(T, HEAD_DIM // 2)

    in_maps = []
    for c in range(N_CORES):
        b, qb = c // 2, c % 2
        qoff = qb * QB
        perm = np.arange(qoff, qoff + QB)
        xb = x[b].reshape(T, D)[perm]
        xT = np.ascontiguousarray(xb.T)              # [D, QB]
        # xA[p, d*512+c] = xT[d*128+p, c]
        xAc = np.ascontiguousarray(
            xT.reshape(DT, 128, QB).transpose(1, 0, 2)
            .reshape(128, DT * QB)).astype(bf)
        in_maps.append({
            "xA": xAc,
            "fqT": np.ascontiguousarray(fq_flat[perm].T),
            "wq4": wq4,
            "wk4": wk4,
            "wvA": wvA,
            "wo4": wo4,
        })
    return in_maps


def kernel(x, freqs, wq, wk, wv, wo, _trace=False, _trace_kwargs=None):
    from concourse.bass_utils import run_bass_kernel_spmd

    x = np.asarray(x, dtype=np.float32)
    freqs = np.asarray(freqs, dtype=np.float32)
    wq = np.asarray(wq, dtype=np.float32)
    wk = np.asarray(wk, dtype=np.float32)
    wv = np.asarray(wv, dtype=np.float32)
    wo = np.asarray(wo, dtype=np.float32)

    if "nc" not in _CACHE:
        _CACHE["nc"] = _build()
    nc = _CACHE["nc"]

    in_maps = _prep_shards(x, freqs, wq, wk, wv, wo)
    res = run_bass_kernel_spmd(
        nc, in_maps, core_ids=list(range(N_CORES)), trace=_trace,
        **(_trace_kwargs or {}))
    _CACHE["last_result"] = res

    full = np.zeros((B, T, D), np.float32)
    for c in range(N_CORES):
        b, qb = c // 2, c % 2
        full[b, qb * QB:(qb + 1) * QB, :] = res.results[c]["out"]
    return full.reshape(B, S, K_POS, D)


# revision 18
# speedup vs baseline: 1.2115x; 1.0271x over previous
"""Trainium2 Bass kernel for GQA attention (B=4, T=1024, D=4096, 32 Q heads,
8 KV heads, RoPE, full softmax attention, output projection).

Sharding: 8 cores = 4 batches x 2 query-blocks of 512 tokens. Each core
computes K/V for the 512 tokens of its own block (pairs of cores that share
a batch exchange halves via 2-rank AllGathers) and runs attention + output
projection for its 512 queries.

Token order per core is host-rotated so the core's query block is always
tokens [0:512) -- full (maskless) attention is permutation-invariant in the
key/value tokens, so each core runs an identical SPMD program.

Datapath: x and all weights are cast to bf16 on the host (halves HBM
traffic) and pre-packed into SBUF-layout [128, N] panels so every weight
DMA is a single contiguous ~1 MiB transfer; all matmuls are bf16 with f32
PSUM accumulation. Per head the softmax denominator comes from a
ones-vector matmul accumulated alongside PV; its reciprocal is broadcast
to 128 partitions via a rank-1 bf16 matmul. Score matmuls+exp are
interleaved into the next head's Q-projection stream so the ScalarE exp
cascade never stalls the PE.

DMA queues: weight/activation loads ride the sync HWDGE ring, SBUF->DRAM
stores ride the scalar HWDGE ring, and the collectives plus gathered K/V
loads ride the gpsimd SWDGE ring, so no compute stream ever queues behind
a collective.
"""

import sys
import math

import numpy as np

if "/opt/trn_rl_repo" not in sys.path:
    sys.path.insert(0, "/opt/trn_rl_repo")

HEAD_DIM = 128
N_HEADS = 32
N_KV = 8
B, S, K_POS, D = 4, 32, 32, 4096
T = S * K_POS          # 1024 tokens per batch
QB = 512               # queries per core
N_CORES = 8
SCALE = HEAD_DIM ** -0.5
DT = D // 128          # 32 d-tiles
LAG = 4                # attention trails Q-projection by LAG heads

_CACHE = {}


def _install_tile_drain_fix():
    """walrus in this image rejects >1 sem wait on one CTRL (Drain)
    instruction; spread the Tile tail-drain waits across sync-engine NOPs."""
    import concourse.tile as tile_mod
    import concourse.mybir as mybir
    from concourse.vector_clock import ScopedClock

    if getattr(tile_mod.TileContext, "_drain_fix_installed", False):
        return

    def _patched(self, tick_clock, wait_clock):
        nc = self.nc
        drain_inst = nc.sync.drain()
        wait_clock.add_sem_waits(
            drain_inst.ins, ScopedClock({None: tick_clock.global_clock})
        )
        si = drain_inst.ins.sync_info
        waits = list(si.on_wait) if si is not None and si.on_wait else []
        if len(waits) > 1:
            si.on_wait = waits[:1]
            for w in waits[1:]:
                nop = nc.sync.nop(nofuse=True)
                nop.ins.sync_info = mybir.SyncInfo(on_wait=[w], on_update=[])
        nc.all_engine_barrier()
        assert self.sems is not None
        popped = nc._tile_sem_poison_stack.pop()
        assert popped is self._sem_poison
        nc.clear_and_free_semaphores(list(self.sems.allocated().values()))
        nc.all_engine_barrier()

    tile_mod.TileContext._drain_and_barrier = _patched
    tile_mod.TileContext._drain_fix_installed = True


def _split_multi_waits(nc, mybir):
    """walrus here rejects >1 sem wait per instruction: hoist extra waits
    onto same-engine NOPs inserted immediately before the instruction."""
    import copy

    template = None
    for fn in nc.m.functions:
        for bb in fn.blocks:
            for inst in bb.instructions:
                if type(inst).__name__ == "InstNoOp":
                    template = inst
                    break
            if template is not None:
                break
    assert template is not None, "no InstNoOp template found"

    n_added = 0
    for fn in nc.m.functions:
        for bb in fn.blocks:
            new_list = []
            changed = False
            for inst in bb.instructions:
                si = inst.sync_info
                waits = list(si.on_wait) if si is not None and si.on_wait else []
                if len(waits) > 1:
                    changed = True
                    for w in waits[:-1]:
                        nop = copy.deepcopy(template)
                        nop.name = f"I-wsplit-{nc.next_id()}"
                        nop.engine = inst.engine
                        nop.sync_info = mybir.SyncInfo(on_wait=[w], on_update=[])
                        nc.register_instruction(nop, overwrite=True)
                        new_list.append(nop)
                        n_added += 1
                    si.on_wait = waits[-1:]
                new_list.append(inst)
            if changed:
                bb.instructions = new_list
    return n_added


def _rope_emit(nc, pool, ps, dst, cos2, sin2, f32):
    """ps: [128, 512] psum (rows 0:64 = even/'real' dims, 64:128 = odd);
    dst: [128, 512] bf16 sbuf. cos2/sin2: [128, 512] with both halves equal
    to cos(f)/sin(f)."""
    a = pool.tile([128, QB], f32, name="rpA", tag="rpA")
    bs = pool.tile([128, QB], f32, name="rpB", tag="rpB")
    nc.vector.tensor_mul(a[:], ps[:], cos2[:])
    nc.vector.tensor_mul(bs[0:64, :], ps[64:128, :], sin2[64:128, :])
    nc.vector.tensor_mul(bs[64:128, :], ps[0:64, :], sin2[0:64, :])
    nc.vector.tensor_sub(dst[0:64, :], a[0:64, :], bs[0:64, :])
    nc.vector.tensor_add(dst[64:128, :], a[64:128, :], bs[64:128, :])


def _build():
    import concourse.bass as bass
    import concourse.mybir as mybir
    import concourse.tile as tile

    _install_tile_drain_fix()

    f32 = mybir.dt.float32
    bf16 = mybir.dt.bfloat16
    Sin = mybir.ActivationFunctionType.Sin
    Exp = mybir.ActivationFunctionType.Exp

    nc = bass.Bass("TRN2", target_bir_lowering=False, debug=False)

    xA = nc.declare_dram_parameter("xA", [128, DT * QB], bf16, isOutput=False)
    fqT = nc.declare_dram_parameter("fqT", [64, QB], f32, isOutput=False)
    wq4 = nc.declare_dram_parameter("wq4", [N_HEADS, 128, D], bf16, isOutput=False)
    wk4 = nc.declare_dram_parameter("wk4", [N_KV, 128, D], bf16, isOutput=False)
    wvA = nc.declare_dram_parameter("wvA", [128, DT * 1024], bf16, isOutput=False)
    wo4 = nc.declare_dram_parameter("wo4", [8, 128, D * 4], bf16, isOutput=False)
    out = nc.declare_dram_parameter("out", [QB, D], f32, isOutput=True)

    rg = [[0, 1], [2, 3], [4, 5], [6, 7]]

    with tile.TileContext(nc) as tc:
        with tc.tile_pool(name="const", bufs=1) as constp:
            # ---- resident tiles (alloc order = reverse release order) ----
            attp = tc.alloc_tile_pool(name="attn", bufs=1)
            attn_sb = [attp.tile([128, QB], bf16, name=f"at{h}")
                       for h in range(N_HEADS)]
            vp = tc.alloc_tile_pool(name="vsb", bufs=1)
            kp = tc.alloc_tile_pool(name="ksb", bufs=1)
            v_sb = [vp.tile([128, T], bf16, name=f"v{kt}") for kt in range(8)]
            k_sb = [kp.tile([128, T], bf16, name=f"k{kh}") for kh in range(N_KV)]
            xqp = tc.alloc_tile_pool(name="xqp", bufs=1)
            xq_all = xqp.tile([128, DT * QB], bf16, name="xq_all")

            def load_xa(j):
                nc.sync.dma_start(
                    out=xq_all[:, j * 4096:(j + 1) * 4096],
                    in_=xA.ap()[:, j * 4096:(j + 1) * 4096])

            def xq_sl(d):
                return xq_all[:, d * QB:(d + 1) * QB]

            wkp = tc.alloc_tile_pool(name="wkp", bufs=2)
            wqp = tc.alloc_tile_pool(name="wqp", bufs=3)
            wop = tc.alloc_tile_pool(name="wop", bufs=2)
            wk_tiles, wq_tiles, wo_tiles = {}, {}, {}

            def emit_wk(kh):
                t = wkp.tile([128, D], bf16, name="wk_sl", tag="wk_sl")
                nc.sync.dma_start(out=t[:], in_=wk4.ap()[kh])
                wk_tiles[kh] = t

            def emit_wq(h):
                t = wqp.tile([128, D], bf16, name="wq_sl", tag="wq_sl")
                nc.sync.dma_start(out=t[:], in_=wq4.ap()[h])
                wq_tiles[h] = t

            def emit_wo(i):
                db, q4 = i // 4, i % 4
                t = wop.tile([128, 8 * QB], bf16, name="wo_sl", tag="wo_sl")
                nc.sync.dma_start(
                    out=t[:], in_=wo4.ap()[db][:, q4 * 4096:(q4 + 1) * 4096])
                wo_tiles[i] = t

            load_xa(0)

            with tc.tile_pool(name="dramb", bufs=1, space="DRAM") as dramp:
                v_half = [dramp.tile([4, 128, QB], bf16, name=f"v_half{fb}")
                          for fb in range(2)]
                v_gath = [dramp.tile([2, 4, 128, QB], bf16, name=f"v_gath{fb}")
                          for fb in range(2)]
                k_half = [dramp.tile([4, 128, QB], bf16, name=f"k_half{i}")
                          for i in range(2)]
                k_gath = [dramp.tile([2, 4, 128, QB], bf16, name=f"k_gath{i}")
                          for i in range(2)]

                # ---- V projection (own 512 tokens): ps[fb*4+tt] = [tok, feat]
                with tc.tile_pool(name="wvp", bufs=3) as wvp, \
                     tc.tile_pool(name="vstg", bufs=1) as vstg, \
                     tc.tile_pool(name="psv", bufs=1, space="PSUM") as psv:
                    wv_tiles = {}

                    def emit_wv(j):
                        t = wvp.tile([128, 4096], bf16, name="wv", tag="wv")
                        nc.sync.dma_start(
                            out=t[:], in_=wvA.ap()[:, j * 4096:(j + 1) * 4096])
                        wv_tiles[j] = t

                    emit_wv(0)

                    # ---- sincos: freqs in [0, 2pi), Sin accepts [-pi, pi]:
                    #   sin(t) = sin(pi - t); cos(t) = 1 - 2*sin(t/2)^2
                    fq_sb = constp.tile([64, QB], f32, name="fq_sb")
                    nc.sync.dma_start(out=fq_sb[:], in_=fqT.ap())
                    load_xa(1)
                    emit_wv(1)
                    load_xa(2)
                    emit_wv(2)
                    load_xa(3)
                    emit_wk(0)
                    cos2 = constp.tile([128, QB], f32, name="cos2")
                    sin2 = constp.tile([128, QB], f32, name="sin2")
                    pi_ap = constp.tile([64, 1], f32, name="pi_ap")
                    nc.vector.memset(pi_ap[:], math.pi)
                    s_half = constp.tile([64, QB], f32, name="s_half")
                    nc.scalar.activation(s_half[:], fq_sb[:], Sin,
                                         bias=0.0, scale=0.5)
                    sq = constp.tile([64, QB], f32, name="sq")
                    nc.vector.tensor_mul(sq[:], s_half[:], s_half[:])
                    for half in (0, 64):
                        nc.vector.tensor_scalar(
                            cos2[half:half + 64, :], sq[:], -2.0, 1.0,
                            mybir.AluOpType.mult, mybir.AluOpType.add)
                        nc.scalar.activation(
                            sin2[half:half + 64, :], fq_sb[:], Sin,
                            bias=pi_ap[:], scale=-1.0)
                    # preload the ScalarE Exp table off the critical path
                    warm = constp.tile([1, 1], f32, name="warm")
                    nc.scalar.activation(warm[:], pi_ap[0:1, 0:1], Exp,
                                         bias=0.0, scale=0.0)

                    ps = [psv.tile([128, QB], f32, name=f"psv{i}",
                                   tag=f"psv{i}") for i in range(8)]
                    for d in range(DT):
                        if d % 4 == 0 and d // 4 + 3 < 8:
                            emit_wv(d // 4 + 3)
                        wv_d = wv_tiles[d // 4]
                        base = (d % 4) * 1024
                        for fb in range(2):
                            for tt in range(4):
                                nc.tensor.matmul(
                                    ps[fb * 4 + tt][:],
                                    lhsT=xq_sl(d)[:, tt * 128:(tt + 1) * 128],
                                    rhs=wv_d[:, base + fb * QB:
                                             base + (fb + 1) * QB],
                                    start=(d == 0), stop=(d == DT - 1))
                        if d // 4 - 1 in wv_tiles and d % 4 == 3:
                            wv_tiles.pop(d // 4 - 1, None)
                    for fb in range(2):
                        for tt in range(4):
                            vs = vstg.tile([128, QB], bf16, name="vs",
                                           tag=f"vs{fb}{tt}")
                            i = fb * 4 + tt
                            if i % 3 == 0:
                                nc.vector.tensor_copy(vs[:], ps[i][:])
                            else:
                                nc.scalar.copy(vs[:], ps[i][:])
                            nc.scalar.dma_start(out=v_half[fb][tt], in_=vs[:])
                        nc.gpsimd.collective_compute(
                            "AllGather", mybir.AluOpType.bypass,
                            ins=[v_half[fb].opt()], outs=[v_gath[fb].opt()],
                            replica_groups=rg)

                # ---- K projection (own 512 tokens) + RoPE ----
                with tc.tile_pool(name="kstg", bufs=2) as kstg, \
                     tc.tile_pool(name="ropek", bufs=2) as ropek, \
                     tc.tile_pool(name="psk", bufs=2, space="PSUM") as psk:
                    for kh in range(N_KV):
                        if kh + 1 < N_KV:
                            emit_wk(kh + 1)
                        if kh >= 5:
                            emit_wq(kh - 5)
                        wk_sl = wk_tiles.pop(kh)
                        pk = psk.tile([128, QB], f32, name="pk", tag="pk")
                        for d in range(DT):
                            nc.tensor.matmul(
                                pk[:],
                                lhsT=wk_sl[:, d * 128:(d + 1) * 128],
                                rhs=xq_sl(d),
                                start=(d == 0), stop=(d == DT - 1))
                        ks = kstg.tile([128, QB], bf16, name="ks", tag="ks")
                        _rope_emit(nc, ropek, pk, ks[:], cos2, sin2, f32)
                        nc.scalar.dma_start(out=k_half[kh // 4][kh % 4],
                                            in_=ks[:])
                        if kh % 4 == 3:
                            nc.gpsimd.collective_compute(
                                "AllGather", mybir.AluOpType.bypass,
                                ins=[k_half[kh // 4].opt()],
                                outs=[k_gath[kh // 4].opt()],
                                replica_groups=rg)

                # gathered K/V -> SBUF (gpsimd ring, behind the collectives)
                for kt in range(8):
                    for fb in range(2):
                        nc.gpsimd.dma_start(
                            out=v_sb[kt][:, fb * QB:(fb + 1) * QB],
                            in_=v_gath[fb][kt // 4, kt % 4])
                for half in range(2):
                    for rr in range(2):
                        for j in range(4):
                            kh = half * 4 + j
                            nc.gpsimd.dma_start(
                                out=k_sb[kh][:, rr * QB:(rr + 1) * QB],
                                in_=k_gath[half][rr, j])

                # ---- Q projection + attention, software-pipelined ----
                _q_attention(nc, tc, mybir, xq_sl, k_sb, v_sb, cos2, sin2,
                             attn_sb, wq_tiles, emit_wq, emit_wo)
                _out_proj(nc, tc, mybir, out, attn_sb, wo_tiles, emit_wo)
                wop.release()
                wqp.release()
                wkp.release()
                xqp.release()
                kp.release()
                vp.release()
                attp.release()

    _split_multi_waits(nc, mybir)
    return nc


def _q_attention(nc, tc, mybir, xq_sl, k_sb, v_sb, cos2, sin2, attn_sb,
                 wq_tiles, emit_wq, emit_wo):
    f32 = mybir.dt.float32
    bf16 = mybir.dt.bfloat16
    Exp = mybir.ActivationFunctionType.Exp

    with tc.tile_pool(name="qsb", bufs=5) as qsb, \
         tc.tile_pool(name="ropeq", bufs=1) as ropeq, \
         tc.tile_pool(name="ptil", bufs=2) as ptp, \
         tc.tile_pool(name="gsum", bufs=1) as gsp, \
         tc.tile_pool(name="ptsum", bufs=2) as tsp, \
         tc.tile_pool(name="pvsb", bufs=3) as pvp, \
         tc.tile_pool(name="rsb", bufs=2) as rsbp, \
         tc.tile_pool(name="pss", bufs=2, space="PSUM") as pss, \
         tc.tile_pool(name="ppv", bufs=1, space="PSUM") as ppv, \
         tc.tile_pool(name="psq", bufs=2, space="PSUM") as psq, \
         tc.tile_pool(name="pden", bufs=2, space="PSUM") as pden, \
         tc.tile_pool(name="prb", bufs=1, space="PSUM") as prb:

        # constant [128,1] / [1,128] ones for the den / broadcast matmuls
        ones_col = qsb.tile([128, 1], bf16, name="ones_col", bufs=1)
        nc.vector.memset(ones_col[:], 1.0)
        ones_row = qsb.tile([1, 128], bf16, name="ones_row", bufs=1)
        nc.vector.memset(ones_row[:], 1.0)

        q_tiles = {}
        St = {}   # a -> stage state dict

        def emit_score_kt(a, kt):
            kh = a // 4
            ps_s = pss.tile([128, QB], f32, name="ps_s", tag="ps_s")
            nc.tensor.matmul(
                ps_s[:], lhsT=k_sb[kh][:, kt * 128:(kt + 1) * 128],
                rhs=q_tiles[a][:], start=True, stop=True)
            nc.scalar.activation(
                St[a]["pt"][:, kt * QB:(kt + 1) * QB], ps_s[:],
                Exp, bias=0.0, scale=SCALE)

        def stage_a_pe(a):
            # PV accumulation (single PSUM bank; evacuated by ScalarE below)
            kh = a // 4
            pt = St[a]["pt"]
            pv = ppv.tile([128, QB], f32, name="pv", tag="pv")
            for kt in range(8):
                nc.tensor.matmul(
                    pv[:], lhsT=v_sb[kt][:, kh * 128:(kh + 1) * 128],
                    rhs=pt[:, kt * QB:(kt + 1) * QB],
                    start=(kt == 0), stop=(kt == 7))
            St[a]["pv"] = pv

        def stage_a_post(a, drain=False):
            # evacuate PV; fold the 8 key-tiles of exp for the denominator.
            # Steady state folds on GpSimd (narrow ops - wide ones contend
            # for SBUF ports); warmup/drain heads fold on DVE, which has
            # slack there while the gpsimd ring is busy with the gathers.
            pt = St[a]["pt"]
            pv_sb = pvp.tile([128, QB], bf16, name="pv_sb", tag="pv_sb")
            if drain:
                nc.vector.tensor_copy(pv_sb[:], St[a]["pv"][:])
            else:
                nc.scalar.copy(pv_sb[:], St[a]["pv"][:])
            ptsum = tsp.tile([128, QB], bf16, name="ptsum", tag="ptsum")
            if drain or a < 3:
                s1 = gsp.tile([128, 2 * QB], bf16, name="dfold", tag="dfold")
                nc.vector.tensor_add(s1[:], pt[:, 0:2 * QB],
                                     pt[:, 2 * QB:4 * QB])
                nc.vector.tensor_add(s1[:], s1[:], pt[:, 4 * QB:6 * QB])
                nc.vector.tensor_add(s1[:], s1[:], pt[:, 6 * QB:8 * QB])
                nc.vector.tensor_add(ptsum[:], s1[:, 0:QB], s1[:, QB:2 * QB])
            else:
                s1 = gsp.tile([128, 2 * QB], bf16, name="dfold", tag="dfold")
                nc.gpsimd.tensor_add(s1[:], pt[:, 0:2 * QB],
                                     pt[:, 2 * QB:4 * QB])
                nc.gpsimd.tensor_add(s1[:, 0:QB], s1[:, 0:QB],
                                     s1[:, QB:2 * QB])
                nc.gpsimd.tensor_add(ptsum[:], pt[:, 4 * QB:5 * QB],
                                     pt[:, 5 * QB:6 * QB])
                nc.gpsimd.tensor_add(ptsum[:], ptsum[:], pt[:, 6 * QB:7 * QB])
                nc.gpsimd.tensor_add(ptsum[:], ptsum[:], pt[:, 7 * QB:8 * QB])
                nc.gpsimd.tensor_add(ptsum[:], ptsum[:], s1[:, 0:QB])
            St[a]["pv_sb"] = pv_sb
            St[a]["ptsum"] = ptsum

        def stage_b_pe(a):
            # den[1,512] = ones.T @ ptsum  (partition reduction on PE)
            den = pden.tile([1, QB], f32, name="den", tag="den")
            nc.tensor.matmul(den[:], lhsT=ones_col[:], rhs=St[a]["ptsum"][:],
                             start=True, stop=True)
            St[a]["den"] = den

        def stage_b_dve(a):
            recip = rsbp.tile([1, QB], bf16, name="recip", tag="recip")
            with nc.allow_low_precision(reason="softmax denom in bf16"):
                nc.vector.reciprocal(recip[:], St[a]["den"][:])
            St[a]["recip"] = recip

        def stage_c_pe(a):
            # broadcast 1/den to 128 partitions (rank-1 bf16 matmul)
            ps_rb = prb.tile([128, QB], f32, name="ps_rb", tag="ps_rb")
            nc.tensor.matmul(ps_rb[:], lhsT=ones_row[:],
                             rhs=St[a]["recip"][:], start=True, stop=True)
            St[a]["ps_rb"] = ps_rb

        def stage_c_post(a):
            rb_sb = rsbp.tile([128, QB], f32, name="rb_sb", tag="rb_sb")
            nc.scalar.copy(rb_sb[:], St[a]["ps_rb"][:])
            nc.vector.tensor_mul(attn_sb[a][:], St[a]["pv_sb"][:], rb_sb[:])

        for h in range(N_HEADS):
            a, b, c = h - LAG, h - LAG - 1, h - LAG - 2
            a = a if a >= 0 else None
            b = b if b >= 0 else None
            c = c if c >= 0 else None

            if h + 3 < N_HEADS:
                emit_wq(h + 3)
            if a is not None:
                St[a] = {"pt": ptp.tile([128, 8 * QB], bf16, name="pt",
                                        tag="pt")}

            ps_q = psq.tile([128, QB], f32, name="ps_q", tag="ps_q")
            wq_sl = wq_tiles.pop(h)
            for d in range(DT):
                nc.tensor.matmul(
                    ps_q[:], lhsT=wq_sl[:, d * 128:(d + 1) * 128],
                    rhs=xq_sl(d), start=(d == 0), stop=(d == DT - 1))
                if a is not None and d % 4 == 3:
                    emit_score_kt(a, d // 4)

            if b is not None:
                stage_b_pe(b)
            if c is not None:
                stage_c_pe(c)
            if a is not None:
                stage_a_pe(a)

            if c is not None:
                stage_c_post(c)
            q_t = qsb.tile([128, QB], bf16, name="q_t", tag="q_t")
            _rope_emit(nc, ropeq, ps_q, q_t[:], cos2, sin2, f32)
            q_tiles[h] = q_t
            if b is not None:
                stage_b_dve(b)
            if a is not None:
                stage_a_post(a)
                q_tiles.pop(a, None)
            if c is not None:
                del St[c]

        # ---- drain: heads 28..31 (scores pair-interleaved for ACT pacing),
        # then flush the b/c stages ----
        emit_wo(0)
        A0, A1 = N_HEADS - LAG, N_HEADS - LAG + 1   # 28, 29
        A2, A3 = N_HEADS - LAG + 2, N_HEADS - LAG + 3
        for a in (A0, A1):
            St[a] = {"pt": ptp.tile([128, 8 * QB], bf16, name="pt", tag="pt")}
        for kt in range(8):
            emit_score_kt(A0, kt)
            emit_score_kt(A1, kt)
        stage_b_pe(A0 - 1)
        stage_c_pe(A0 - 2)
        stage_a_pe(A0)
        stage_a_pe(A1)
        stage_c_post(A0 - 2)
        stage_b_dve(A0 - 1)
        stage_a_post(A0, drain=True)
        stage_a_post(A1, drain=True)
        q_tiles.pop(A0, None)
        q_tiles.pop(A1, None)
        del St[A0 - 2]

        for a in (A2, A3):
            St[a] = {"pt": ptp.tile([128, 8 * QB], bf16, name="pt", tag="pt")}
        for kt in range(8):
            emit_score_kt(A2, kt)
            emit_score_kt(A3, kt)
        stage_b_pe(A0)
        stage_b_pe(A1)
        stage_c_pe(A0 - 1)
        stage_a_pe(A2)
        stage_a_pe(A3)
        stage_c_post(A0 - 1)
        stage_b_dve(A0)
        stage_b_dve(A1)
        stage_a_post(A2, drain=True)
        stage_a_post(A3, drain=True)
        q_tiles.pop(A2, None)
        q_tiles.pop(A3, None)
        del St[A0 - 1]

        stage_b_pe(A2)
        stage_b_pe(A3)
        stage_c_pe(A0)
        stage_c_pe(A1)
        stage_c_post(A0)
        stage_c_post(A1)
        stage_b_dve(A2)
        stage_b_dve(A3)
        del St[A0], St[A1]

        stage_c_pe(A2)
        stage_c_pe(A3)
        stage_c_post(A2)
        stage_c_post(A3)
        del St[A2], St[A3]


def _out_proj(nc, tc, mybir, out, attn_sb, wo_tiles, emit_wo):
    f32 = mybir.dt.float32
    with tc.tile_pool(name="psout", bufs=2, space="PSUM") as psout, \
         tc.tile_pool(name="ostg", bufs=4) as ostg:

        for db in range(8):
            po = [psout.tile([128, QB], f32, name=f"po{qt}", tag=f"po{qt}")
                  for qt in range(4)]
            for q4 in range(4):
                i = db * 4 + q4
                if i + 1 < 32:
                    emit_wo(i + 1)
                wo_sl = wo_tiles.pop(i)
                for f8 in range(8):
                    f = q4 * 8 + f8
                    for qt in range(4):
                        nc.tensor.matmul(
                            po[qt][:],
                            lhsT=attn_sb[f][:, qt * 128:(qt + 1) * 128],
                            rhs=wo_sl[:, f8 * QB:(f8 + 1) * QB],
                            start=(f == 0), stop=(f == 31))
            for qt in range(4):
                o_stg = ostg.tile([128, QB], f32, name="o_stg", tag="o_stg")
                nc.vector.tensor_copy(o_stg[:], po[qt][:])
                nc.scalar.dma_start(
                    out=out.ap()[qt * 128:(qt + 1) * 128,
                                 db * QB:(db + 1) * QB],
                    in_=o_stg[:])


def _prep_shards(x, freqs, wq, wk, wv, wo):
    """Host-side sharding + layout prep + bf16 cast (numpy only)."""
    import ml_dtypes
    bf = ml_dtypes.bfloat16

    rope_perm = np.concatenate([np.arange(0, HEAD_DIM, 2),
                                np.arange(1, HEAD_DIM, 2)])
    f_perm_q = np.concatenate([h * HEAD_DIM + rope_perm for h in range(N_HEADS)])
    f_perm_k = np.concatenate([h * HEAD_DIM + rope_perm for h in range(N_KV)])

    wqT_p = np.ascontiguousarray(wq[f_perm_q].T)     # [D, 4096]
    wkT_p = np.ascontiguousarray(wk[f_perm_k].T)     # [D, 1024]
    wvT = np.ascontiguousarray(wv.T)                 # [D, 1024]
    woT = wo.T                                        # [F, D]

    # wq4[h, p, d*128+c] = wqT_p[d*128+p, h*128+c]
    wq4 = np.ascontiguousarray(
        wqT_p.reshape(DT, 128, N_HEADS, 128).transpose(2, 1, 0, 3)
        .reshape(N_HEADS, 128, D)).astype(bf)
    wk4 = np.ascontiguousarray(
        wkT_p.reshape(DT, 128, N_KV, 128).transpose(2, 1, 0, 3)
        .reshape(N_KV, 128, D)).astype(bf)
    # wvA[p, d*1024+f] = wvT[d*128+p, f]
    wvA = np.ascontiguousarray(
        wvT.reshape(DT, 128, N_KV * 128).transpose(1, 0, 2)
        .reshape(128, DT * 1024)).astype(bf)
    # wo4[db, fp, ft*512+c] = woT[ft*128+fp, db*512+c]
    wo4 = np.ascontiguousarray(
        woT.reshape(DT, 128, 8, 512).transpose(2, 1, 0, 3)
        .reshape(8, 128, D * 4)).astype(bf)

    fq_flat = freqs.reshape(T, HEAD_DIM // 2)

    in_maps = []
    for c in range(N_CORES):
        b, qb = c // 2, c % 2
        qoff = qb * QB
        perm = np.arange(qoff, qoff + QB)
        xb = x[b].reshape(T, D)[perm]
        xT = np.ascontiguousarray(xb.T)              # [D, QB]
        # xA[p, d*512+c] = xT[d*128+p, c]
        xAc = np.ascontiguousarray(
            xT.reshape(DT, 128, QB).transpose(1, 0, 2)
            .reshape(128, DT * QB)).astype(bf)
        in_maps.append({
            "xA": xAc,
            "fqT": np.ascontiguousarray(fq_flat[perm].T),
            "wq4": wq4,
            "wk4": wk4,
            "wvA": wvA,
            "wo4": wo4,
        })
    return in_maps


def kernel(x, freqs, wq, wk, wv, wo, _trace=False, _trace_kwargs=None):
    from concourse.bass_utils import run_bass_kernel_spmd

    x = np.asarray(x, dtype=np.float32)
    freqs = np.asarray(freqs, dtype=np.float32)
    wq = np.asarray(wq, dtype=np.float32)
    wk = np.asarray(wk, dtype=np.float32)
    wv = np.asarray(wv, dtype=np.float32)
    wo = np.asarray(wo, dtype=np.float32)

    if "nc" not in _CACHE:
        _CACHE["nc"] = _build()
    nc = _CACHE["nc"]

    in_maps = _prep_shards(x, freqs, wq, wk, wv, wo)
    res = run_bass_kernel_spmd(
        nc, in_maps, core_ids=list(range(N_CORES)), trace=_trace,
        **(_trace_kwargs or {}))
    _CACHE["last_result"] = res

    full = np.zeros((B, T, D), np.float32)
    for c in range(N_CORES):
        b, qb = c // 2, c % 2
        full[b, qb * QB:(qb + 1) * QB, :] = res.results[c]["out"]
    return full.reshape(B, S, K_POS, D)


# revision 20
# speedup vs baseline: 1.2338x; 1.0184x over previous
"""Trainium2 Bass kernel for GQA attention (B=4, T=1024, D=4096, 32 Q heads,
8 KV heads, RoPE, full softmax attention, output projection).

Sharding: 8 cores = 4 batches x 2 query-blocks of 512 tokens. Each core
computes K/V for the 512 tokens of its own block (pairs of cores that share
a batch exchange halves via 2-rank AllGathers) and runs attention + output
projection for its 512 queries.

Token order per core is host-rotated so the core's query block is always
tokens [0:512) -- full (maskless) attention is permutation-invariant in the
key/value tokens, so each core runs an identical SPMD program.

Datapath: x and all weights are cast to bf16 on the host (halves HBM
traffic) and pre-packed into SBUF-layout [128, N] panels so every weight
DMA is a single contiguous ~1 MiB transfer; all matmuls are bf16 with f32
PSUM accumulation. Per head the softmax denominator comes from a
ones-vector matmul accumulated alongside PV; its reciprocal is broadcast
to 128 partitions via a rank-1 bf16 matmul. Score matmuls+exp are
interleaved into the next head's Q-projection stream so the ScalarE exp
cascade never stalls the PE.

DMA queues: weight/activation loads ride the sync HWDGE ring, SBUF->DRAM
stores ride the scalar HWDGE ring, and the collectives plus gathered K/V
loads ride the gpsimd SWDGE ring, so no compute stream ever queues behind
a collective.
"""

import sys
import math

import numpy as np

if "/opt/trn_rl_repo" not in sys.path:
    sys.path.insert(0, "/opt/trn_rl_repo")

HEAD_DIM = 128
N_HEADS = 32
N_KV = 8
B, S, K_POS, D = 4, 32, 32, 4096
T = S * K_POS          # 1024 tokens per batch
QB = 512               # queries per core
N_CORES = 8
SCALE = HEAD_DIM ** -0.5
DT = D // 128          # 32 d-tiles
LAG = 4                # attention trails Q-projection by LAG heads

_CACHE = {}


def _install_tile_drain_fix():
    """walrus in this image rejects >1 sem wait on one CTRL (Drain)
    instruction; spread the Tile tail-drain waits across sync-engine NOPs."""
    import concourse.tile as tile_mod
    import concourse.mybir as mybir
    from concourse.vector_clock import ScopedClock

    if getattr(tile_mod.TileContext, "_drain_fix_installed", False):
        return

    def _patched(self, tick_clock, wait_clock):
        nc = self.nc
        drain_inst = nc.sync.drain()
        wait_clock.add_sem_waits(
            drain_inst.ins, ScopedClock({None: tick_clock.global_clock})
        )
        si = drain_inst.ins.sync_info
        waits = list(si.on_wait) if si is not None and si.on_wait else []
        if len(waits) > 1:
            si.on_wait = waits[:1]
            for w in waits[1:]:
                nop = nc.sync.nop(nofuse=True)
                nop.ins.sync_info = mybir.SyncInfo(on_wait=[w], on_update=[])
        nc.all_engine_barrier()
        assert self.sems is not None
        popped = nc._tile_sem_poison_stack.pop()
        assert popped is self._sem_poison
        nc.clear_and_free_semaphores(list(self.sems.allocated().values()))
        nc.all_engine_barrier()

    tile_mod.TileContext._drain_and_barrier = _patched
    tile_mod.TileContext._drain_fix_installed = True


def _split_multi_waits(nc, mybir):
    """walrus here rejects >1 sem wait per instruction: hoist extra waits
    onto same-engine NOPs inserted immediately before the instruction."""
    import copy

    template = None
    for fn in nc.m.functions:
        for bb in fn.blocks:
            for inst in bb.instructions:
                if type(inst).__name__ == "InstNoOp":
                    template = inst
                    break
            if template is not None:
                break
    assert template is not None, "no InstNoOp template found"

    n_added = 0
    for fn in nc.m.functions:
        for bb in fn.blocks:
            new_list = []
            changed = False
            for inst in bb.instructions:
                si = inst.sync_info
                waits = list(si.on_wait) if si is not None and si.on_wait else []
                if len(waits) > 1:
                    changed = True
                    for w in waits[:-1]:
                        nop = copy.deepcopy(template)
                        nop.name = f"I-wsplit-{nc.next_id()}"
                        nop.engine = inst.engine
                        nop.sync_info = mybir.SyncInfo(on_wait=[w], on_update=[])
                        nc.register_instruction(nop, overwrite=True)
                        new_list.append(nop)
                        n_added += 1
                    si.on_wait = waits[-1:]
                new_list.append(inst)
            if changed:
                bb.instructions = new_list
    return n_added


def _rope_emit(nc, pool, ps, dst, cos2, sin2, f32):
    """ps: [128, 512] psum (rows 0:64 = even/'real' dims, 64:128 = odd);
    dst: [128, 512] bf16 sbuf. cos2/sin2: [128, 512] with both halves equal
    to cos(f)/sin(f)."""
    a = pool.tile([128, QB], f32, name="rpA", tag="rpA")
    bs = pool.tile([128, QB], f32, name="rpB", tag="rpB")
    nc.vector.tensor_mul(a[:], ps[:], cos2[:])
    nc.vector.tensor_mul(bs[0:64, :], ps[64:128, :], sin2[64:128, :])
    nc.vector.tensor_mul(bs[64:128, :], ps[0:64, :], sin2[0:64, :])
    nc.vector.tensor_sub(dst[0:64, :], a[0:64, :], bs[0:64, :])
    nc.vector.tensor_add(dst[64:128, :], a[64:128, :], bs[64:128, :])


def _build():
    import concourse.bass as bass
    import concourse.mybir as mybir
    import concourse.tile as tile

    _install_tile_drain_fix()

    f32 = mybir.dt.float32
    bf16 = mybir.dt.bfloat16
    Sin = mybir.ActivationFunctionType.Sin
    Exp = mybir.ActivationFunctionType.Exp

    nc = bass.Bass("TRN2", target_bir_lowering=False, debug=False)

    xA = nc.declare_dram_parameter("xA", [128, DT * QB], bf16, isOutput=False)
    fqT = nc.declare_dram_parameter("fqT", [64, QB], f32, isOutput=False)
    wq4 = nc.declare_dram_parameter("wq4", [N_HEADS, 128, D], bf16, isOutput=False)
    wk4 = nc.declare_dram_parameter("wk4", [N_KV, 128, D], bf16, isOutput=False)
    wvA = nc.declare_dram_parameter("wvA", [128, DT * 1024], bf16, isOutput=False)
    wo4 = nc.declare_dram_parameter("wo4", [8, 128, D * 4], bf16, isOutput=False)
    out = nc.declare_dram_parameter("out", [QB, D], f32, isOutput=True)

    rg = [[0, 1], [2, 3], [4, 5], [6, 7]]

    with tile.TileContext(nc) as tc:
        with tc.tile_pool(name="const", bufs=1) as constp:
            # ---- resident tiles (alloc order = reverse release order) ----
            attp = tc.alloc_tile_pool(name="attn", bufs=1)
            attn_sb = [attp.tile([128, QB], bf16, name=f"at{h}")
                       for h in range(N_HEADS)]
            vp = tc.alloc_tile_pool(name="vsb", bufs=1)
            kp = tc.alloc_tile_pool(name="ksb", bufs=1)
            v_sb = [vp.tile([128, T], bf16, name=f"v{kt}") for kt in range(8)]
            k_sb = [kp.tile([128, T], bf16, name=f"k{kh}") for kh in range(N_KV)]
            xqp = tc.alloc_tile_pool(name="xqp", bufs=1)
            xq_all = xqp.tile([128, DT * QB], bf16, name="xq_all")

            def load_xa(j):
                nc.sync.dma_start(
                    out=xq_all[:, j * 4096:(j + 1) * 4096],
                    in_=xA.ap()[:, j * 4096:(j + 1) * 4096])

            def xq_sl(d):
                return xq_all[:, d * QB:(d + 1) * QB]

            wkp = tc.alloc_tile_pool(name="wkp", bufs=2)
            wqp = tc.alloc_tile_pool(name="wqp", bufs=3)
            wop = tc.alloc_tile_pool(name="wop", bufs=2)
            wk_tiles, wq_tiles, wo_tiles = {}, {}, {}

            def emit_wk(kh):
                t = wkp.tile([128, D], bf16, name="wk_sl", tag="wk_sl")
                nc.sync.dma_start(out=t[:], in_=wk4.ap()[kh])
                wk_tiles[kh] = t

            def emit_wq(h):
                t = wqp.tile([128, D], bf16, name="wq_sl", tag="wq_sl")
                nc.sync.dma_start(out=t[:], in_=wq4.ap()[h])
                wq_tiles[h] = t

            def emit_wo(i):
                db, q4 = i // 4, i % 4
                t = wop.tile([128, 8 * QB], bf16, name="wo_sl", tag="wo_sl")
                nc.sync.dma_start(
                    out=t[:], in_=wo4.ap()[db][:, q4 * 4096:(q4 + 1) * 4096])
                wo_tiles[i] = t

            nc.sync.dma_start(out=xq_all[:, 0:1024],
                              in_=xA.ap()[:, 0:1024])
            nc.sync.dma_start(out=xq_all[:, 1024:4096],
                              in_=xA.ap()[:, 1024:4096])

            with tc.tile_pool(name="dramb", bufs=1, space="DRAM") as dramp:
                v_half = [dramp.tile([4, 128, QB], bf16, name=f"v_half{fb}")
                          for fb in range(2)]
                v_gath = [dramp.tile([2, 4, 128, QB], bf16, name=f"v_gath{fb}")
                          for fb in range(2)]
                k_half = [dramp.tile([4, 128, QB], bf16, name=f"k_half{i}")
                          for i in range(2)]
                k_gath = [dramp.tile([2, 4, 128, QB], bf16, name=f"k_gath{i}")
                          for i in range(2)]

                # ---- V projection (own 512 tokens): ps[tt] = [tok, feat],
                # fb-outer so fb0's AllGather launches mid-V ----
                with tc.tile_pool(name="wvp", bufs=3) as wvp, \
                     tc.tile_pool(name="vstg", bufs=1) as vstg, \
                     tc.tile_pool(name="psv", bufs=1, space="PSUM") as psv:
                    wv_tiles = {}

                    def emit_wv(fb, j):
                        t = wvp.tile([128, 4096], bf16, name="wv", tag="wv")
                        nc.sync.dma_start(
                            out=t[:],
                            in_=wvA.ap()[:, fb * 16384 + j * 4096:
                                         fb * 16384 + (j + 1) * 4096])
                        wv_tiles[(fb, j)] = t

                    # fast first slice: d=0..7 of fb0
                    emit_wv(0, 0)

                    # ---- sincos: freqs in [0, 2pi), Sin accepts [-pi, pi]:
                    #   sin(t) = sin(pi - t); cos(t) = 1 - 2*sin(t/2)^2
                    fq_sb = constp.tile([64, QB], f32, name="fq_sb")
                    nc.sync.dma_start(out=fq_sb[:], in_=fqT.ap())
                    load_xa(1)
                    emit_wv(0, 1)
                    load_xa(2)
                    emit_wv(0, 2)
                    load_xa(3)
                    emit_wk(0)
                    cos2 = constp.tile([128, QB], f32, name="cos2")
                    sin2 = constp.tile([128, QB], f32, name="sin2")
                    pi_ap = constp.tile([64, 1], f32, name="pi_ap")
                    nc.vector.memset(pi_ap[:], math.pi)
                    s_half = constp.tile([64, QB], f32, name="s_half")
                    nc.scalar.activation(s_half[:], fq_sb[:], Sin,
                                         bias=0.0, scale=0.5)
                    sq = constp.tile([64, QB], f32, name="sq")
                    nc.vector.tensor_mul(sq[:], s_half[:], s_half[:])
                    for half in (0, 64):
                        nc.vector.tensor_scalar(
                            cos2[half:half + 64, :], sq[:], -2.0, 1.0,
                            mybir.AluOpType.mult, mybir.AluOpType.add)
                        nc.scalar.activation(
                            sin2[half:half + 64, :], fq_sb[:], Sin,
                            bias=pi_ap[:], scale=-1.0)
                    # preload the ScalarE Exp table off the critical path
                    warm = constp.tile([1, 1], f32, name="warm")
                    nc.scalar.activation(warm[:], pi_ap[0:1, 0:1], Exp,
                                         bias=0.0, scale=0.0)

                    for fb in range(2):
                        ps = [psv.tile([128, QB], f32, name=f"psv{tt}",
                                       tag=f"psv{tt}") for tt in range(4)]
                        for d in range(DT):
                            j = d // 8
                            if d % 8 == 0 and (fb, j + 1) not in wv_tiles:
                                nj = j + 1
                                nfb = fb
                                if nj > 3:
                                    nj, nfb = 0, fb + 1
                                if nfb < 2:
                                    emit_wv(nfb, nj)
                            wv_d = wv_tiles[(fb, j)]
                            base = (d % 8) * QB
                            for tt in range(4):
                                nc.tensor.matmul(
                                    ps[tt][:],
                                    lhsT=xq_sl(d)[:, tt * 128:(tt + 1) * 128],
                                    rhs=wv_d[:, base:base + QB],
                                    start=(d == 0), stop=(d == DT - 1))
                            if d % 8 == 7:
                                wv_tiles.pop((fb, j), None)
                        for tt in range(4):
                            vs = vstg.tile([128, QB], bf16, name="vs",
                                           tag=f"vs{fb}{tt}")
                            if tt % 2 == 0:
                                nc.vector.tensor_copy(vs[:], ps[tt][:])
                            else:
                                nc.scalar.copy(vs[:], ps[tt][:])
                            nc.scalar.dma_start(out=v_half[fb][tt], in_=vs[:])
                        nc.gpsimd.collective_compute(
                            "AllGather", mybir.AluOpType.bypass,
                            ins=[v_half[fb].opt()], outs=[v_gath[fb].opt()],
                            replica_groups=rg)

                # ---- K projection (own 512 tokens) + RoPE ----
                with tc.tile_pool(name="kstg", bufs=2) as kstg, \
                     tc.tile_pool(name="ropek", bufs=2) as ropek, \
                     tc.tile_pool(name="psk", bufs=2, space="PSUM") as psk:
                    for kh in range(N_KV):
                        if kh + 1 < N_KV:
                            emit_wk(kh + 1)
                        if kh >= 5:
                            emit_wq(kh - 5)
                        wk_sl = wk_tiles.pop(kh)
                        pk = psk.tile([128, QB], f32, name="pk", tag="pk")
                        for d in range(DT):
                            nc.tensor.matmul(
                                pk[:],
                                lhsT=wk_sl[:, d * 128:(d + 1) * 128],
                                rhs=xq_sl(d),
                                start=(d == 0), stop=(d == DT - 1))
                        ks = kstg.tile([128, QB], bf16, name="ks", tag="ks")
                        _rope_emit(nc, ropek, pk, ks[:], cos2, sin2, f32)
                        nc.scalar.dma_start(out=k_half[kh // 4][kh % 4],
                                            in_=ks[:])
                        if kh % 4 == 3:
                            nc.gpsimd.collective_compute(
                                "AllGather", mybir.AluOpType.bypass,
                                ins=[k_half[kh // 4].opt()],
                                outs=[k_gath[kh // 4].opt()],
                                replica_groups=rg)

                # gathered K/V -> SBUF (gpsimd ring, behind the collectives)
                for kt in range(8):
                    for fb in range(2):
                        nc.gpsimd.dma_start(
                            out=v_sb[kt][:, fb * QB:(fb + 1) * QB],
                            in_=v_gath[fb][kt // 4, kt % 4])
                for half in range(2):
                    for rr in range(2):
                        for j in range(4):
                            kh = half * 4 + j
                            nc.gpsimd.dma_start(
                                out=k_sb[kh][:, rr * QB:(rr + 1) * QB],
                                in_=k_gath[half][rr, j])

                # ---- Q projection + attention, software-pipelined ----
                _q_attention(nc, tc, mybir, xq_sl, k_sb, v_sb, cos2, sin2,
                             attn_sb, wq_tiles, emit_wq, emit_wo)
                _out_proj(nc, tc, mybir, out, attn_sb, wo_tiles, emit_wo)
                wop.release()
                wqp.release()
                wkp.release()
                xqp.release()
                kp.release()
                vp.release()
                attp.release()

    _split_multi_waits(nc, mybir)
    return nc


def _q_attention(nc, tc, mybir, xq_sl, k_sb, v_sb, cos2, sin2, attn_sb,
                 wq_tiles, emit_wq, emit_wo):
    f32 = mybir.dt.float32
    bf16 = mybir.dt.bfloat16
    Exp = mybir.ActivationFunctionType.Exp

    with tc.tile_pool(name="qsb", bufs=5) as qsb, \
         tc.tile_pool(name="ropeq", bufs=1) as ropeq, \
         tc.tile_pool(name="ptil", bufs=2) as ptp, \
         tc.tile_pool(name="gsum", bufs=1) as gsp, \
         tc.tile_pool(name="ptsum", bufs=2) as tsp, \
         tc.tile_pool(name="pvsb", bufs=3) as pvp, \
         tc.tile_pool(name="rsb", bufs=2) as rsbp, \
         tc.tile_pool(name="pss", bufs=2, space="PSUM") as pss, \
         tc.tile_pool(name="ppv", bufs=1, space="PSUM") as ppv, \
         tc.tile_pool(name="psq", bufs=2, space="PSUM") as psq, \
         tc.tile_pool(name="pden", bufs=2, space="PSUM") as pden, \
         tc.tile_pool(name="prb", bufs=1, space="PSUM") as prb:

        # constant [128,1] / [1,128] ones for the den / broadcast matmuls
        ones_col = qsb.tile([128, 1], bf16, name="ones_col", bufs=1)
        nc.vector.memset(ones_col[:], 1.0)
        ones_row = qsb.tile([1, 128], bf16, name="ones_row", bufs=1)
        nc.vector.memset(ones_row[:], 1.0)

        q_tiles = {}
        St = {}   # a -> stage state dict

        def emit_score_kt(a, kt):
            kh = a // 4
            ps_s = pss.tile([128, QB], f32, name="ps_s", tag="ps_s")
            nc.tensor.matmul(
                ps_s[:], lhsT=k_sb[kh][:, kt * 128:(kt + 1) * 128],
                rhs=q_tiles[a][:], start=True, stop=True)
            nc.scalar.activation(
                St[a]["pt"][:, kt * QB:(kt + 1) * QB], ps_s[:],
                Exp, bias=0.0, scale=SCALE)

        def stage_a_pe(a):
            # PV accumulation (single PSUM bank; evacuated by ScalarE below)
            kh = a // 4
            pt = St[a]["pt"]
            pv = ppv.tile([128, QB], f32, name="pv", tag="pv")
            for kt in range(8):
                nc.tensor.matmul(
                    pv[:], lhsT=v_sb[kt][:, kh * 128:(kh + 1) * 128],
                    rhs=pt[:, kt * QB:(kt + 1) * QB],
                    start=(kt == 0), stop=(kt == 7))
            St[a]["pv"] = pv

        def stage_a_post(a, drain=False):
            # evacuate PV; fold the 8 key-tiles of exp for the denominator.
            # Steady state folds on GpSimd (narrow ops - wide ones contend
            # for SBUF ports); warmup/drain heads fold on DVE, which has
            # slack there while the gpsimd ring is busy with the gathers.
            pt = St[a]["pt"]
            pv_sb = pvp.tile([128, QB], bf16, name="pv_sb", tag="pv_sb")
            if drain:
                nc.vector.tensor_copy(pv_sb[:], St[a]["pv"][:])
            else:
                nc.scalar.copy(pv_sb[:], St[a]["pv"][:])
            ptsum = tsp.tile([128, QB], bf16, name="ptsum", tag="ptsum")
            if drain or a < 6:
                s1 = gsp.tile([128, 2 * QB], bf16, name="dfold", tag="dfold")
                nc.vector.tensor_add(s1[:], pt[:, 0:2 * QB],
                                     pt[:, 2 * QB:4 * QB])
                nc.vector.tensor_add(s1[:], s1[:], pt[:, 4 * QB:6 * QB])
                nc.vector.tensor_add(s1[:], s1[:], pt[:, 6 * QB:8 * QB])
                nc.vector.tensor_add(ptsum[:], s1[:, 0:QB], s1[:, QB:2 * QB])
            else:
                s1 = gsp.tile([128, 2 * QB], bf16, name="dfold", tag="dfold")
                nc.gpsimd.tensor_add(s1[:], pt[:, 0:2 * QB],
                                     pt[:, 2 * QB:4 * QB])
                nc.gpsimd.tensor_add(s1[:, 0:QB], s1[:, 0:QB],
                                     s1[:, QB:2 * QB])
                nc.gpsimd.tensor_add(ptsum[:], pt[:, 4 * QB:5 * QB],
                                     pt[:, 5 * QB:6 * QB])
                nc.gpsimd.tensor_add(ptsum[:], ptsum[:], pt[:, 6 * QB:7 * QB])
                nc.gpsimd.tensor_add(ptsum[:], ptsum[:], pt[:, 7 * QB:8 * QB])
                nc.gpsimd.tensor_add(ptsum[:], ptsum[:], s1[:, 0:QB])
            St[a]["pv_sb"] = pv_sb
            St[a]["ptsum"] = ptsum

        def stage_b_pe(a):
            # den[1,512] = ones.T @ ptsum  (partition reduction on PE)
            den = pden.tile([1, QB], f32, name="den", tag="den")
            nc.tensor.matmul(den[:], lhsT=ones_col[:], rhs=St[a]["ptsum"][:],
                             start=True, stop=True)
            St[a]["den"] = den

        def stage_b_dve(a):
            recip = rsbp.tile([1, QB], bf16, name="recip", tag="recip")
            with nc.allow_low_precision(reason="softmax denom in bf16"):
                nc.vector.reciprocal(recip[:], St[a]["den"][:])
            St[a]["recip"] = recip

        def stage_c_pe(a):
            # broadcast 1/den to 128 partitions (rank-1 bf16 matmul)
            ps_rb = prb.tile([128, QB], f32, name="ps_rb", tag="ps_rb")
            nc.tensor.matmul(ps_rb[:], lhsT=ones_row[:],
                             rhs=St[a]["recip"][:], start=True, stop=True)
            St[a]["ps_rb"] = ps_rb

        def stage_c_post(a):
            rb_sb = rsbp.tile([128, QB], f32, name="rb_sb", tag="rb_sb")
            nc.scalar.copy(rb_sb[:], St[a]["ps_rb"][:])
            nc.vector.tensor_mul(attn_sb[a][:], St[a]["pv_sb"][:], rb_sb[:])

        for h in range(N_HEADS):
            a, b, c = h - LAG, h - LAG - 1, h - LAG - 2
            a = a if a >= 0 else None
            b = b if b >= 0 else None
            c = c if c >= 0 else None

            if h + 3 < N_HEADS:
                emit_wq(h + 3)
            if a is not None:
                St[a] = {"pt": ptp.tile([128, 8 * QB], bf16, name="pt",
                                        tag="pt")}

            ps_q = psq.tile([128, QB], f32, name="ps_q", tag="ps_q")
            wq_sl = wq_tiles.pop(h)
            for d in range(DT):
                nc.tensor.matmul(
                    ps_q[:], lhsT=wq_sl[:, d * 128:(d + 1) * 128],
                    rhs=xq_sl(d), start=(d == 0), stop=(d == DT - 1))
                if a is not None and d % 4 == 3:
                    emit_score_kt(a, d // 4)

            if b is not None:
                stage_b_pe(b)
            if c is not None:
                stage_c_pe(c)
            if a is not None:
                stage_a_pe(a)

            if c is not None:
                stage_c_post(c)
            q_t = qsb.tile([128, QB], bf16, name="q_t", tag="q_t")
            _rope_emit(nc, ropeq, ps_q, q_t[:], cos2, sin2, f32)
            q_tiles[h] = q_t
            if b is not None:
                stage_b_dve(b)
            if a is not None:
                stage_a_post(a)
                q_tiles.pop(a, None)
            if c is not None:
                del St[c]

        # ---- drain: heads 28..31 (scores pair-interleaved for ACT pacing),
        # then flush the b/c stages ----
        emit_wo(0)
        A0, A1 = N_HEADS - LAG, N_HEADS - LAG + 1   # 28, 29
        A2, A3 = N_HEADS - LAG + 2, N_HEADS - LAG + 3
        for a in (A0, A1):
            St[a] = {"pt": ptp.tile([128, 8 * QB], bf16, name="pt", tag="pt")}
        for kt in range(8):
            emit_score_kt(A0, kt)
            emit_score_kt(A1, kt)
        stage_b_pe(A0 - 1)
        stage_c_pe(A0 - 2)
        stage_a_pe(A0)
        stage_a_pe(A1)
        stage_c_post(A0 - 2)
        stage_b_dve(A0 - 1)
        stage_a_post(A0)
        stage_a_post(A1, drain=True)
        q_tiles.pop(A0, None)
        q_tiles.pop(A1, None)
        del St[A0 - 2]

        for a in (A2, A3):
            St[a] = {"pt": ptp.tile([128, 8 * QB], bf16, name="pt", tag="pt")}
        for kt in range(8):
            emit_score_kt(A2, kt)
            emit_score_kt(A3, kt)
        stage_b_pe(A0)
        stage_b_pe(A1)
        stage_c_pe(A0 - 1)
        stage_a_pe(A2)
        stage_a_pe(A3)
        stage_c_post(A0 - 1)
        stage_b_dve(A0)
        stage_b_dve(A1)
        stage_a_post(A2)
        stage_a_post(A3, drain=True)
        q_tiles.pop(A2, None)
        q_tiles.pop(A3, None)
        del St[A0 - 1]

        stage_b_pe(A2)
        stage_b_pe(A3)
        stage_c_pe(A0)
        stage_c_pe(A1)
        stage_c_post(A0)
        stage_c_post(A1)
        stage_b_dve(A2)
        stage_b_dve(A3)
        del St[A0], St[A1]

        stage_c_pe(A2)
        stage_c_pe(A3)
        stage_c_post(A2)
        stage_c_post(A3)
        del St[A2], St[A3]


def _out_proj(nc, tc, mybir, out, attn_sb, wo_tiles, emit_wo):
    f32 = mybir.dt.float32
    with tc.tile_pool(name="psout", bufs=2, space="PSUM") as psout, \
         tc.tile_pool(name="ostg", bufs=4) as ostg:

        for db in range(8):
            po = [psout.tile([128, QB], f32, name=f"po{qt}", tag=f"po{qt}")
                  for qt in range(4)]
            for q4 in range(4):
                i = db * 4 + q4
                if i + 1 < 32:
                    emit_wo(i + 1)
                wo_sl = wo_tiles.pop(i)
                for f8 in range(8):
                    f = q4 * 8 + f8
                    for qt in range(4):
                        nc.tensor.matmul(
                            po[qt][:],
                            lhsT=attn_sb[f][:, qt * 128:(qt + 1) * 128],
                            rhs=wo_sl[:, f8 * QB:(f8 + 1) * QB],
                            start=(f == 0), stop=(f == 31))
            for qt in range(4):
                o_stg = ostg.tile([128, QB], f32, name="o_stg", tag="o_stg")
                nc.vector.tensor_copy(o_stg[:], po[qt][:])
                nc.scalar.dma_start(
                    out=out.ap()[qt * 128:(qt + 1) * 128,
                                 db * QB:(db + 1) * QB],
                    in_=o_stg[:])


def _prep_shards(x, freqs, wq, wk, wv, wo):
    """Host-side sharding + layout prep + bf16 cast (numpy only)."""
    import ml_dtypes
    bf = ml_dtypes.bfloat16

    rope_perm = np.concatenate([np.arange(0, HEAD_DIM, 2),
                                np.arange(1, HEAD_DIM, 2)])
    f_perm_q = np.concatenate([h * HEAD_DIM + rope_perm for h in range(N_HEADS)])
    f_perm_k = np.concatenate([h * HEAD_DIM + rope_perm for h in range(N_KV)])

    wqT_p = np.ascontiguousarray(wq[f_perm_q].T)     # [D, 4096]
    wkT_p = np.ascontiguousarray(wk[f_perm_k].T)     # [D, 1024]
    wvT = np.ascontiguousarray(wv.T)                 # [D, 1024]
    woT = wo.T                                        # [F, D]

    # wq4[h, p, d*128+c] = wqT_p[d*128+p, h*128+c]
    wq4 = np.ascontiguousarray(
        wqT_p.reshape(DT, 128, N_HEADS, 128).transpose(2, 1, 0, 3)
        .reshape(N_HEADS, 128, D)).astype(bf)
    wk4 = np.ascontiguousarray(
        wkT_p.reshape(DT, 128, N_KV, 128).transpose(2, 1, 0, 3)
        .reshape(N_KV, 128, D)).astype(bf)
    # wvA[p, fb*16384 + d*512 + c] = wvT[d*128+p, fb*512+c]
    wvA = np.ascontiguousarray(
        wvT.reshape(DT, 128, 2, 512).transpose(1, 2, 0, 3)
        .reshape(128, DT * 1024)).astype(bf)
    # wo4[db, fp, ft*512+c] = woT[ft*128+fp, db*512+c]
    wo4 = np.ascontiguousarray(
        woT.reshape(DT, 128, 8, 512).transpose(2, 1, 0, 3)
        .reshape(8, 128, D * 4)).astype(bf)

    fq_flat = freqs.reshape(T, HEAD_DIM // 2)

    in_maps = []
    for c in range(N_CORES):
        b, qb = c // 2, c % 2
        qoff = qb * QB
        perm = np.arange(qoff, qoff + QB)
        xb = x[b].reshape(T, D)[perm]
        xT = np.ascontiguousarray(xb.T)              # [D, QB]
        # xA[p, d*512+c] = xT[d*128+p, c]
        xAc = np.ascontiguousarray(
            xT.reshape(DT, 128, QB).transpose(1, 0, 2)
            .reshape(128, DT * QB)).astype(bf)
        in_maps.append({
            "xA": xAc,
            "fqT": np.ascontiguousarray(fq_flat[perm].T),
            "wq4": wq4,
            "wk4": wk4,
            "wvA": wvA,
            "wo4": wo4,
        })
    return in_maps


def kernel(x, freqs, wq, wk, wv, wo, _trace=False, _trace_kwargs=None):
    from concourse.bass_utils import run_bass_kernel_spmd

    x = np.asarray(x, dtype=np.float32)
    freqs = np.asarray(freqs, dtype=np.float32)
    wq = np.asarray(wq, dtype=np.float32)
    wk = np.asarray(wk, dtype=np.float32)
    wv = np.asarray(wv, dtype=np.float32)
    wo = np.asarray(wo, dtype=np.float32)

    if "nc" not in _CACHE:
        _CACHE["nc"] = _build()
    nc = _CACHE["nc"]

    in_maps = _prep_shards(x, freqs, wq, wk, wv, wo)
    res = run_bass_kernel_spmd(
        nc, in_maps, core_ids=list(range(N_CORES)), trace=_trace,
        **(_trace_kwargs or {}))
    _CACHE["last_result"] = res

    full = np.zeros((B, T, D), np.float32)
    for c in range(N_CORES):
        b, qb = c // 2, c % 2
        full[b, qb * QB:(qb + 1) * QB, :] = res.results[c]["out"]
    return full.reshape(B, S, K_POS, D)


# revision 21
# speedup vs baseline: 1.2363x; 1.0020x over previous
"""Trainium2 Bass kernel for GQA attention (B=4, T=1024, D=4096, 32 Q heads,
8 KV heads, RoPE, full softmax attention, output projection).

Sharding: 8 cores = 4 batches x 2 query-blocks of 512 tokens. Each core
computes K/V for the 512 tokens of its own block (pairs of cores that share
a batch exchange halves via 2-rank AllGathers) and runs attention + output
projection for its 512 queries.

Token order per core is host-rotated so the core's query block is always
tokens [0:512) -- full (maskless) attention is permutation-invariant in the
key/value tokens, so each core runs an identical SPMD program.

Datapath: x and all weights are cast to bf16 on the host (halves HBM
traffic) and pre-packed into SBUF-layout [128, N] panels so every weight
DMA is a single contiguous ~1 MiB transfer; all matmuls are bf16 with f32
PSUM accumulation. Per head the softmax denominator comes from a
ones-vector matmul accumulated alongside PV; its reciprocal is broadcast
to 128 partitions via a rank-1 bf16 matmul. Score matmuls+exp are
interleaved into the next head's Q-projection stream so the ScalarE exp
cascade never stalls the PE.

DMA queues: weight/activation loads ride the sync HWDGE ring, SBUF->DRAM
stores ride the scalar HWDGE ring, and the collectives plus gathered K/V
loads ride the gpsimd SWDGE ring, so no compute stream ever queues behind
a collective.
"""

import sys
import math

import numpy as np

if "/opt/trn_rl_repo" not in sys.path:
    sys.path.insert(0, "/opt/trn_rl_repo")

HEAD_DIM = 128
N_HEADS = 32
N_KV = 8
B, S, K_POS, D = 4, 32, 32, 4096
T = S * K_POS          # 1024 tokens per batch
QB = 512               # queries per core
N_CORES = 8
SCALE = HEAD_DIM ** -0.5
DT = D // 128          # 32 d-tiles
LAG = 2                # attention trails Q-projection by LAG heads

_CACHE = {}


def _install_tile_drain_fix():
    """walrus in this image rejects >1 sem wait on one CTRL (Drain)
    instruction; spread the Tile tail-drain waits across sync-engine NOPs."""
    import concourse.tile as tile_mod
    import concourse.mybir as mybir
    from concourse.vector_clock import ScopedClock

    if getattr(tile_mod.TileContext, "_drain_fix_installed", False):
        return

    def _patched(self, tick_clock, wait_clock):
        nc = self.nc
        drain_inst = nc.sync.drain()
        wait_clock.add_sem_waits(
            drain_inst.ins, ScopedClock({None: tick_clock.global_clock})
        )
        si = drain_inst.ins.sync_info
        waits = list(si.on_wait) if si is not None and si.on_wait else []
        if len(waits) > 1:
            si.on_wait = waits[:1]
            for w in waits[1:]:
                nop = nc.sync.nop(nofuse=True)
                nop.ins.sync_info = mybir.SyncInfo(on_wait=[w], on_update=[])
        nc.all_engine_barrier()
        assert self.sems is not None
        popped = nc._tile_sem_poison_stack.pop()
        assert popped is self._sem_poison
        nc.clear_and_free_semaphores(list(self.sems.allocated().values()))
        nc.all_engine_barrier()

    tile_mod.TileContext._drain_and_barrier = _patched
    tile_mod.TileContext._drain_fix_installed = True


def _split_multi_waits(nc, mybir):
    """walrus here rejects >1 sem wait per instruction: hoist extra waits
    onto same-engine NOPs inserted immediately before the instruction."""
    import copy

    template = None
    for fn in nc.m.functions:
        for bb in fn.blocks:
            for inst in bb.instructions:
                if type(inst).__name__ == "InstNoOp":
                    template = inst
                    break
            if template is not None:
                break
    assert template is not None, "no InstNoOp template found"

    n_added = 0
    for fn in nc.m.functions:
        for bb in fn.blocks:
            new_list = []
            changed = False
            for inst in bb.instructions:
                si = inst.sync_info
                waits = list(si.on_wait) if si is not None and si.on_wait else []
                if len(waits) > 1:
                    changed = True
                    for w in waits[:-1]:
                        nop = copy.deepcopy(template)
                        nop.name = f"I-wsplit-{nc.next_id()}"
                        nop.engine = inst.engine
                        nop.sync_info = mybir.SyncInfo(on_wait=[w], on_update=[])
                        nc.register_instruction(nop, overwrite=True)
                        new_list.append(nop)
                        n_added += 1
                    si.on_wait = waits[-1:]
                new_list.append(inst)
            if changed:
                bb.instructions = new_list
    return n_added


def _rope_emit(nc, pool, ps, dst, cos2, sin2, f32):
    """ps: [128, 512] psum (rows 0:64 = even/'real' dims, 64:128 = odd);
    dst: [128, 512] bf16 sbuf. cos2/sin2: [128, 512] with both halves equal
    to cos(f)/sin(f)."""
    a = pool.tile([128, QB], f32, name="rpA", tag="rpA")
    bs = pool.tile([128, QB], f32, name="rpB", tag="rpB")
    nc.vector.tensor_mul(a[:], ps[:], cos2[:])
    nc.vector.tensor_mul(bs[0:64, :], ps[64:128, :], sin2[64:128, :])
    nc.vector.tensor_mul(bs[64:128, :], ps[0:64, :], sin2[0:64, :])
    nc.vector.tensor_sub(dst[0:64, :], a[0:64, :], bs[0:64, :])
    nc.vector.tensor_add(dst[64:128, :], a[64:128, :], bs[64:128, :])


def _build():
    import concourse.bass as bass
    import concourse.mybir as mybir
    import concourse.tile as tile

    _install_tile_drain_fix()

    f32 = mybir.dt.float32
    bf16 = mybir.dt.bfloat16
    Sin = mybir.ActivationFunctionType.Sin
    Exp = mybir.ActivationFunctionType.Exp

    nc = bass.Bass("TRN2", target_bir_lowering=False, debug=False)

    xA = nc.declare_dram_parameter("xA", [128, DT * QB], bf16, isOutput=False)
    fqT = nc.declare_dram_parameter("fqT", [64, QB], f32, isOutput=False)
    wq4 = nc.declare_dram_parameter("wq4", [N_HEADS, 128, D], bf16, isOutput=False)
    wk4 = nc.declare_dram_parameter("wk4", [N_KV, 128, D], bf16, isOutput=False)
    wvA = nc.declare_dram_parameter("wvA", [128, DT * 1024], bf16, isOutput=False)
    wo4 = nc.declare_dram_parameter("wo4", [8, 128, D * 4], bf16, isOutput=False)
    out = nc.declare_dram_parameter("out", [QB, D], f32, isOutput=True)

    rg = [[0, 1], [2, 3], [4, 5], [6, 7]]

    with tile.TileContext(nc) as tc:
        with tc.tile_pool(name="const", bufs=1) as constp:
            # ---- resident tiles (alloc order = reverse release order) ----
            attp = tc.alloc_tile_pool(name="attn", bufs=1)
            attn_sb = [attp.tile([128, QB], bf16, name=f"at{h}")
                       for h in range(N_HEADS)]
            vp = tc.alloc_tile_pool(name="vsb", bufs=1)
            kp = tc.alloc_tile_pool(name="ksb", bufs=1)
            v_sb = [vp.tile([128, T], bf16, name=f"v{kt}") for kt in range(8)]
            k_sb = [kp.tile([128, T], bf16, name=f"k{kh}") for kh in range(N_KV)]
            xqp = tc.alloc_tile_pool(name="xqp", bufs=1)
            xq_all = xqp.tile([128, DT * QB], bf16, name="xq_all")

            def load_xa(j):
                nc.sync.dma_start(
                    out=xq_all[:, j * 4096:(j + 1) * 4096],
                    in_=xA.ap()[:, j * 4096:(j + 1) * 4096])

            def xq_sl(d):
                return xq_all[:, d * QB:(d + 1) * QB]

            wkp = tc.alloc_tile_pool(name="wkp", bufs=2)
            wqp = tc.alloc_tile_pool(name="wqp", bufs=3)
            wop = tc.alloc_tile_pool(name="wop", bufs=2)
            wk_tiles, wq_tiles, wo_tiles = {}, {}, {}

            def emit_wk(kh):
                t = wkp.tile([128, D], bf16, name="wk_sl", tag="wk_sl")
                nc.sync.dma_start(out=t[:], in_=wk4.ap()[kh])
                wk_tiles[kh] = t

            def emit_wq(h):
                t = wqp.tile([128, D], bf16, name="wq_sl", tag="wq_sl")
                nc.sync.dma_start(out=t[:], in_=wq4.ap()[h])
                wq_tiles[h] = t

            def emit_wo(i):
                db, q4 = i // 4, i % 4
                t = wop.tile([128, 8 * QB], bf16, name="wo_sl", tag="wo_sl")
                nc.sync.dma_start(
                    out=t[:], in_=wo4.ap()[db][:, q4 * 4096:(q4 + 1) * 4096])
                wo_tiles[i] = t

            nc.sync.dma_start(out=xq_all[:, 0:1024],
                              in_=xA.ap()[:, 0:1024])
            nc.sync.dma_start(out=xq_all[:, 1024:4096],
                              in_=xA.ap()[:, 1024:4096])

            with tc.tile_pool(name="dramb", bufs=1, space="DRAM") as dramp:
                v_half = [dramp.tile([4, 128, QB], bf16, name=f"v_half{fb}")
                          for fb in range(2)]
                v_gath = [dramp.tile([2, 4, 128, QB], bf16, name=f"v_gath{fb}")
                          for fb in range(2)]
                k_half = [dramp.tile([4, 128, QB], bf16, name=f"k_half{i}")
                          for i in range(2)]
                k_gath = [dramp.tile([2, 4, 128, QB], bf16, name=f"k_gath{i}")
                          for i in range(2)]

                # ---- V projection (own 512 tokens): ps[tt] = [tok, feat],
                # fb-outer so fb0's AllGather launches mid-V ----
                with tc.tile_pool(name="wvp", bufs=3) as wvp, \
                     tc.tile_pool(name="vstg", bufs=1) as vstg, \
                     tc.tile_pool(name="psv", bufs=1, space="PSUM") as psv:
                    wv_tiles = {}

                    def emit_wv(fb, j, split=False):
                        t = wvp.tile([128, 4096], bf16, name="wv", tag="wv")
                        base = fb * 16384 + j * 4096
                        if split:
                            nc.sync.dma_start(
                                out=t[:, 0:1024],
                                in_=wvA.ap()[:, base:base + 1024])
                            nc.sync.dma_start(
                                out=t[:, 1024:4096],
                                in_=wvA.ap()[:, base + 1024:base + 4096])
                        else:
                            nc.sync.dma_start(
                                out=t[:], in_=wvA.ap()[:, base:base + 4096])
                        wv_tiles[(fb, j)] = t

                    # fast first slice: d=0..1 of fb0 land early
                    emit_wv(0, 0, split=True)

                    # ---- sincos: freqs in [0, 2pi), Sin accepts [-pi, pi]:
                    #   sin(t) = sin(pi - t); cos(t) = 1 - 2*sin(t/2)^2
                    fq_sb = constp.tile([64, QB], f32, name="fq_sb")
                    nc.sync.dma_start(out=fq_sb[:], in_=fqT.ap())
                    load_xa(1)
                    emit_wv(0, 1)
                    load_xa(2)
                    emit_wv(0, 2)
                    load_xa(3)
                    emit_wk(0)
                    cos2 = constp.tile([128, QB], f32, name="cos2")
                    sin2 = constp.tile([128, QB], f32, name="sin2")
                    pi_ap = constp.tile([64, 1], f32, name="pi_ap")
                    nc.vector.memset(pi_ap[:], math.pi)
                    s_half = constp.tile([64, QB], f32, name="s_half")
                    nc.scalar.activation(s_half[:], fq_sb[:], Sin,
                                         bias=0.0, scale=0.5)
                    sq = constp.tile([64, QB], f32, name="sq")
                    nc.vector.tensor_mul(sq[:], s_half[:], s_half[:])
                    for half in (0, 64):
                        nc.vector.tensor_scalar(
                            cos2[half:half + 64, :], sq[:], -2.0, 1.0,
                            mybir.AluOpType.mult, mybir.AluOpType.add)
                        nc.scalar.activation(
                            sin2[half:half + 64, :], fq_sb[:], Sin,
                            bias=pi_ap[:], scale=-1.0)
                    # preload the ScalarE Exp table off the critical path
                    warm = constp.tile([1, 1], f32, name="warm")
                    nc.scalar.activation(warm[:], pi_ap[0:1, 0:1], Exp,
                                         bias=0.0, scale=0.0)

                    for fb in range(2):
                        ps = [psv.tile([128, QB], f32, name=f"psv{tt}",
                                       tag=f"psv{tt}") for tt in range(4)]
                        for d in range(DT):
                            j = d // 8
                            if d % 8 == 0 and (fb, j + 1) not in wv_tiles:
                                nj = j + 1
                                nfb = fb
                                if nj > 3:
                                    nj, nfb = 0, fb + 1
                                if nfb < 2:
                                    emit_wv(nfb, nj)
                            wv_d = wv_tiles[(fb, j)]
                            base = (d % 8) * QB
                            for tt in range(4):
                                nc.tensor.matmul(
                                    ps[tt][:],
                                    lhsT=xq_sl(d)[:, tt * 128:(tt + 1) * 128],
                                    rhs=wv_d[:, base:base + QB],
                                    start=(d == 0), stop=(d == DT - 1))
                            if d % 8 == 7:
                                wv_tiles.pop((fb, j), None)
                        for tt in range(4):
                            vs = vstg.tile([128, QB], bf16, name="vs",
                                           tag=f"vs{fb}{tt}")
                            if tt % 2 == 0:
                                nc.vector.tensor_copy(vs[:], ps[tt][:])
                            else:
                                nc.scalar.copy(vs[:], ps[tt][:])
                            nc.scalar.dma_start(out=v_half[fb][tt], in_=vs[:])
                        nc.gpsimd.collective_compute(
                            "AllGather", mybir.AluOpType.bypass,
                            ins=[v_half[fb].opt()], outs=[v_gath[fb].opt()],
                            replica_groups=rg)

                # ---- K projection (own 512 tokens) + RoPE ----
                with tc.tile_pool(name="kstg", bufs=2) as kstg, \
                     tc.tile_pool(name="ropek", bufs=2) as ropek, \
                     tc.tile_pool(name="psk", bufs=2, space="PSUM") as psk:
                    for kh in range(N_KV):
                        if kh + 1 < N_KV:
                            emit_wk(kh + 1)
                        if kh >= 5:
                            emit_wq(kh - 5)
                        wk_sl = wk_tiles.pop(kh)
                        pk = psk.tile([128, QB], f32, name="pk", tag="pk")
                        for d in range(DT):
                            nc.tensor.matmul(
                                pk[:],
                                lhsT=wk_sl[:, d * 128:(d + 1) * 128],
                                rhs=xq_sl(d),
                                start=(d == 0), stop=(d == DT - 1))
                        ks = kstg.tile([128, QB], bf16, name="ks", tag="ks")
                        _rope_emit(nc, ropek, pk, ks[:], cos2, sin2, f32)
                        nc.scalar.dma_start(out=k_half[kh // 4][kh % 4],
                                            in_=ks[:])
                        if kh % 4 == 3:
                            nc.gpsimd.collective_compute(
                                "AllGather", mybir.AluOpType.bypass,
                                ins=[k_half[kh // 4].opt()],
                                outs=[k_gath[kh // 4].opt()],
                                replica_groups=rg)

                # gathered K/V -> SBUF (gpsimd ring, behind the collectives)
                for kt in range(8):
                    for fb in range(2):
                        nc.gpsimd.dma_start(
                            out=v_sb[kt][:, fb * QB:(fb + 1) * QB],
                            in_=v_gath[fb][kt // 4, kt % 4])
                for half in range(2):
                    for rr in range(2):
                        for j in range(4):
                            kh = half * 4 + j
                            nc.gpsimd.dma_start(
                                out=k_sb[kh][:, rr * QB:(rr + 1) * QB],
                                in_=k_gath[half][rr, j])

                # ---- Q projection + attention, software-pipelined ----
                _q_attention(nc, tc, mybir, xq_sl, k_sb, v_sb, cos2, sin2,
                             attn_sb, wq_tiles, emit_wq, emit_wo)
                _out_proj(nc, tc, mybir, out, attn_sb, wo_tiles, emit_wo)
                wop.release()
                wqp.release()
                wkp.release()
                xqp.release()
                kp.release()
                vp.release()
                attp.release()

    _split_multi_waits(nc, mybir)
    return nc


def _q_attention(nc, tc, mybir, xq_sl, k_sb, v_sb, cos2, sin2, attn_sb,
                 wq_tiles, emit_wq, emit_wo):
    f32 = mybir.dt.float32
    bf16 = mybir.dt.bfloat16
    Exp = mybir.ActivationFunctionType.Exp

    with tc.tile_pool(name="qsb", bufs=4) as qsb, \
         tc.tile_pool(name="ropeq", bufs=1) as ropeq, \
         tc.tile_pool(name="ptil", bufs=2) as ptp, \
         tc.tile_pool(name="gsum", bufs=1) as gsp, \
         tc.tile_pool(name="ptsum", bufs=2) as tsp, \
         tc.tile_pool(name="pvsb", bufs=3) as pvp, \
         tc.tile_pool(name="rsb", bufs=2) as rsbp, \
         tc.tile_pool(name="pss", bufs=2, space="PSUM") as pss, \
         tc.tile_pool(name="ppv", bufs=1, space="PSUM") as ppv, \
         tc.tile_pool(name="psq", bufs=2, space="PSUM") as psq, \
         tc.tile_pool(name="pden", bufs=2, space="PSUM") as pden, \
         tc.tile_pool(name="prb", bufs=1, space="PSUM") as prb:

        # constant [128,1] / [1,128] ones for the den / broadcast matmuls
        ones_col = qsb.tile([128, 1], bf16, name="ones_col", bufs=1)
        nc.vector.memset(ones_col[:], 1.0)
        ones_row = qsb.tile([1, 128], bf16, name="ones_row", bufs=1)
        nc.vector.memset(ones_row[:], 1.0)

        q_tiles = {}
        St = {}   # a -> stage state dict

        def emit_score_kt(a, kt):
            kh = a // 4
            ps_s = pss.tile([128, QB], f32, name="ps_s", tag="ps_s")
            nc.tensor.matmul(
                ps_s[:], lhsT=k_sb[kh][:, kt * 128:(kt + 1) * 128],
                rhs=q_tiles[a][:], start=True, stop=True)
            nc.scalar.activation(
                St[a]["pt"][:, kt * QB:(kt + 1) * QB], ps_s[:],
                Exp, bias=0.0, scale=SCALE)

        def stage_a_pe(a):
            # PV accumulation (single PSUM bank; evacuated by ScalarE below)
            kh = a // 4
            pt = St[a]["pt"]
            pv = ppv.tile([128, QB], f32, name="pv", tag="pv")
            for kt in range(8):
                nc.tensor.matmul(
                    pv[:], lhsT=v_sb[kt][:, kh * 128:(kh + 1) * 128],
                    rhs=pt[:, kt * QB:(kt + 1) * QB],
                    start=(kt == 0), stop=(kt == 7))
            St[a]["pv"] = pv

        def stage_a_post(a, drain=False):
            # evacuate PV; fold the 8 key-tiles of exp for the denominator.
            # Steady state folds on GpSimd (narrow ops - wide ones contend
            # for SBUF ports); warmup/drain heads fold on DVE, which has
            # slack there while the gpsimd ring is busy with the gathers.
            pt = St[a]["pt"]
            pv_sb = pvp.tile([128, QB], bf16, name="pv_sb", tag="pv_sb")
            if drain:
                nc.vector.tensor_copy(pv_sb[:], St[a]["pv"][:])
            else:
                nc.scalar.copy(pv_sb[:], St[a]["pv"][:])
            ptsum = tsp.tile([128, QB], bf16, name="ptsum", tag="ptsum")
            if drain or a < 6:
                s1 = gsp.tile([128, 2 * QB], bf16, name="dfold", tag="dfold")
                nc.vector.tensor_add(s1[:], pt[:, 0:2 * QB],
                                     pt[:, 2 * QB:4 * QB])
                nc.vector.tensor_add(s1[:], s1[:], pt[:, 4 * QB:6 * QB])
                nc.vector.tensor_add(s1[:], s1[:], pt[:, 6 * QB:8 * QB])
                nc.vector.tensor_add(ptsum[:], s1[:, 0:QB], s1[:, QB:2 * QB])
            else:
                s1 = gsp.tile([128, 2 * QB], bf16, name="dfold", tag="dfold")
                nc.gpsimd.tensor_add(s1[:], pt[:, 0:2 * QB],
                                     pt[:, 2 * QB:4 * QB])
                nc.gpsimd.tensor_add(s1[:, 0:QB], s1[:, 0:QB],
                                     s1[:, QB:2 * QB])
                nc.gpsimd.tensor_add(ptsum[:], pt[:, 4 * QB:5 * QB],
                                     pt[:, 5 * QB:6 * QB])
                nc.gpsimd.tensor_add(ptsum[:], ptsum[:], pt[:, 6 * QB:7 * QB])
                nc.gpsimd.tensor_add(ptsum[:], ptsum[:], pt[:, 7 * QB:8 * QB])
                nc.gpsimd.tensor_add(ptsum[:], ptsum[:], s1[:, 0:QB])
            St[a]["pv_sb"] = pv_sb
            St[a]["ptsum"] = ptsum

        def stage_b_pe(a):
            # den[1,512] = ones.T @ ptsum  (partition reduction on PE)
            den = pden.tile([1, QB], f32, name="den", tag="den")
            nc.tensor.matmul(den[:], lhsT=ones_col[:], rhs=St[a]["ptsum"][:],
                             start=True, stop=True)
            St[a]["den"] = den

        def stage_b_dve(a):
            recip = rsbp.tile([1, QB], bf16, name="recip", tag="recip")
            with nc.allow_low_precision(reason="softmax denom in bf16"):
                nc.vector.reciprocal(recip[:], St[a]["den"][:])
            St[a]["recip"] = recip

        def stage_c_pe(a):
            # broadcast 1/den to 128 partitions (rank-1 bf16 matmul)
            ps_rb = prb.tile([128, QB], f32, name="ps_rb", tag="ps_rb")
            nc.tensor.matmul(ps_rb[:], lhsT=ones_row[:],
                             rhs=St[a]["recip"][:], start=True, stop=True)
            St[a]["ps_rb"] = ps_rb

        def stage_c_post(a):
            rb_sb = rsbp.tile([128, QB], f32, name="rb_sb", tag="rb_sb")
            nc.scalar.copy(rb_sb[:], St[a]["ps_rb"][:])
            nc.vector.tensor_mul(attn_sb[a][:], St[a]["pv_sb"][:], rb_sb[:])

        for h in range(N_HEADS):
            a, b, c = h - LAG, h - LAG - 1, h - LAG - 2
            a = a if a >= 0 else None
            b = b if b >= 0 else None
            c = c if c >= 0 else None

            if h + 3 < N_HEADS:
                emit_wq(h + 3)
            if a is not None:
                St[a] = {"pt": ptp.tile([128, 8 * QB], bf16, name="pt",
                                        tag="pt")}

            ps_q = psq.tile([128, QB], f32, name="ps_q", tag="ps_q")
            wq_sl = wq_tiles.pop(h)
            for d in range(DT):
                nc.tensor.matmul(
                    ps_q[:], lhsT=wq_sl[:, d * 128:(d + 1) * 128],
                    rhs=xq_sl(d), start=(d == 0), stop=(d == DT - 1))
                if a is not None and d % 4 == 3:
                    emit_score_kt(a, d // 4)

            if b is not None:
                stage_b_pe(b)
            if c is not None:
                stage_c_pe(c)
            if a is not None:
                stage_a_pe(a)

            if c is not None:
                stage_c_post(c)
            q_t = qsb.tile([128, QB], bf16, name="q_t", tag="q_t")
            _rope_emit(nc, ropeq, ps_q, q_t[:], cos2, sin2, f32)
            q_tiles[h] = q_t
            if b is not None:
                stage_b_dve(b)
            if a is not None:
                stage_a_post(a)
                q_tiles.pop(a, None)
            if c is not None:
                del St[c]

        # ---- drain: heads 30..31 (scores pair-interleaved for ACT
        # pacing), then flush the b/c stages ----
        emit_wo(0)
        A0, A1 = N_HEADS - LAG, N_HEADS - LAG + 1   # 30, 31
        for a in (A0, A1):
            St[a] = {"pt": ptp.tile([128, 8 * QB], bf16, name="pt", tag="pt")}
        for kt in range(8):
            emit_score_kt(A0, kt)
            emit_score_kt(A1, kt)
        stage_b_pe(A0 - 1)
        stage_c_pe(A0 - 2)
        stage_a_pe(A0)
        stage_a_pe(A1)
        stage_c_post(A0 - 2)
        stage_b_dve(A0 - 1)
        stage_a_post(A0)
        stage_a_post(A1, drain=True)
        q_tiles.pop(A0, None)
        q_tiles.pop(A1, None)
        del St[A0 - 2]

        stage_b_pe(A0)
        stage_b_pe(A1)
        stage_c_pe(A0 - 1)
        stage_c_post(A0 - 1)
        stage_b_dve(A0)
        stage_b_dve(A1)
        del St[A0 - 1]

        stage_c_pe(A0)
        stage_c_post(A0)
        stage_c_pe(A1)
        stage_c_post(A1)
        del St[A0], St[A1]

def _out_proj(nc, tc, mybir, out, attn_sb, wo_tiles, emit_wo):
    f32 = mybir.dt.float32
    with tc.tile_pool(name="psout", bufs=2, space="PSUM") as psout, \
         tc.tile_pool(name="ostg", bufs=4) as ostg:

        for db in range(8):
            po = [psout.tile([128, QB], f32, name=f"po{qt}", tag=f"po{qt}")
                  for qt in range(4)]
            for q4 in range(4):
                i = db * 4 + q4
                if i + 1 < 32:
                    emit_wo(i + 1)
                wo_sl = wo_tiles.pop(i)
                for f8 in range(8):
                    f = q4 * 8 + f8
                    for qt in range(4):
                        nc.tensor.matmul(
                            po[qt][:],
                            lhsT=attn_sb[f][:, qt * 128:(qt + 1) * 128],
                            rhs=wo_sl[:, f8 * QB:(f8 + 1) * QB],
                            start=(f == 0), stop=(f == 31))
            for qt in range(4):
                o_stg = ostg.tile([128, QB], f32, name="o_stg", tag="o_stg")
                nc.vector.tensor_copy(o_stg[:], po[qt][:])
                nc.scalar.dma_start(
                    out=out.ap()[qt * 128:(qt + 1) * 128,
                                 db * QB:(db + 1) * QB],
                    in_=o_stg[:])


def _prep_shards(x, freqs, wq, wk, wv, wo):
    """Host-side sharding + layout prep + bf16 cast (numpy only)."""
    import ml_dtypes
    bf = ml_dtypes.bfloat16

    rope_perm = np.concatenate([np.arange(0, HEAD_DIM, 2),
                                np.arange(1, HEAD_DIM, 2)])
    f_perm_q = np.concatenate([h * HEAD_DIM + rope_perm for h in range(N_HEADS)])
    f_perm_k = np.concatenate([h * HEAD_DIM + rope_perm for h in range(N_KV)])

    wqT_p = np.ascontiguousarray(wq[f_perm_q].T)     # [D, 4096]
    wkT_p = np.ascontiguousarray(wk[f_perm_k].T)     # [D, 1024]
    wvT = np.ascontiguousarray(wv.T)                 # [D, 1024]
    woT = wo.T                                        # [F, D]

    # wq4[h, p, d*128+c] = wqT_p[d*128+p, h*128+c]
    wq4 = np.ascontiguousarray(
        wqT_p.reshape(DT, 128, N_HEADS, 128).transpose(2, 1, 0, 3)
        .reshape(N_HEADS, 128, D)).astype(bf)
    wk4 = np.ascontiguousarray(
        wkT_p.reshape(DT, 128, N_KV, 128).transpose(2, 1, 0, 3)
        .reshape(N_KV, 128, D)).astype(bf)
    # wvA[p, fb*16384 + d*512 + c] = wvT[d*128+p, fb*512+c]
    wvA = np.ascontiguousarray(
        wvT.reshape(DT, 128, 2, 512).transpose(1, 2, 0, 3)
        .reshape(128, DT * 1024)).astype(bf)
    # wo4[db, fp, ft*512+c] = woT[ft*128+fp, db*512+c]
    wo4 = np.ascontiguousarray(
        woT.reshape(DT, 128, 8, 512).transpose(2, 1, 0, 3)
        .reshape(8, 128, D * 4)).astype(bf)

    fq_flat = freqs.reshape(T, HEAD_DIM // 2)

    in_maps = []
    for c in range(N_CORES):
        b, qb = c // 2, c % 2
        qoff = qb * QB
        perm = np.arange(qoff, qoff + QB)
        xb = x[b].reshape(T, D)[perm]
        xT = np.ascontiguousarray(xb.T)              # [D, QB]
        # xA[p, d*512+c] = xT[d*128+p, c]
        xAc = np.ascontiguousarray(
            xT.reshape(DT, 128, QB).transpose(1, 0, 2)
            .reshape(128, DT * QB)).astype(bf)
        in_maps.append({
            "xA": xAc,
            "fqT": np.ascontiguousarray(fq_flat[perm].T),
            "wq4": wq4,
            "wk4": wk4,
            "wvA": wvA,
            "wo4": wo4,
        })
    return in_maps


def kernel(x, freqs, wq, wk, wv, wo, _trace=False, _trace_kwargs=None):
    from concourse.bass_utils import run_bass_kernel_spmd

    x = np.asarray(x, dtype=np.float32)
    freqs = np.asarray(freqs, dtype=np.float32)
    wq = np.asarray(wq, dtype=np.float32)
    wk = np.asarray(wk, dtype=np.float32)
    wv = np.asarray(wv, dtype=np.float32)
    wo = np.asarray(wo, dtype=np.float32)

    if "nc" not in _CACHE:
        _CACHE["nc"] = _build()
    nc = _CACHE["nc"]

    in_maps = _prep_shards(x, freqs, wq, wk, wv, wo)
    res = run_bass_kernel_spmd(
        nc, in_maps, core_ids=list(range(N_CORES)), trace=_trace,
        **(_trace_kwargs or {}))
    _CACHE["last_result"] = res

    full = np.zeros((B, T, D), np.float32)
    for c in range(N_CORES):
        b, qb = c // 2, c % 2
        full[b, qb * QB:(qb + 1) * QB, :] = res.results[c]["out"]
    return full.reshape(B, S, K_POS, D)
